# revision 1
# baseline (speedup 1.0000x reference)
"""GAT 2-layer message-passing network on 8 TRN2 NeuronCores (Bass/Tile).

v2: dma_gather-based (HW indirect_dma_start only supports 1 idx/partition).

Strategy (dst-sharded):
 - Host: add self loops, sort edges by dst, shard dst-node ranges across cores.
   Each core owns nodes [c*NPC, (c+1)*NPC) and ALL edges into them.
 - Edge slots: per dst-block of 128 nodes, edges sub-grouped by src chunk
   (4 chunks of CH rows so int16 indices work), each (block,chunk) run padded
   to x128 slots = tiles. Superblocks of SBG blocks share gather calls.
 - Phase A (replicated): full feature table htab[n] = [h|a_src|pad] bf16
   [Np, 384] (768B rows for dma_gather), + local stats table stats_loc
   [NPCp, 128] bf16 rows [a_dst(H)|pad] for the core's own nodes (from xT_loc).
 - Phase B (L1): per sb: dma_gather htab rows by src (4 chunk calls) +
   stats_loc rows by local dst (1 call); ex = exp(lrelu(asrc+adst)) batched
   per sb; msg in-place in gather buffer ([h*ex|ex|ex] in cols 0:264);
   one-hot from dloc vs iota; per-block PSUM matmul accumulation over its
   tiles; normalize by summed ex, +b1, relu; h2aug = relu @ W2aug via PE
   transpose; write h2loc (AG input) + h2pad (local gather table).
 - AllGather h2loc -> h2tab [N,4] f32; repack into h2tabp [Npp, 64] f32 rows.
 - Phase C (L2): same slots: gather h2tabp by src (4 chunk calls) + h2pad by
   local dst; 4-wide bf16 messages; one-hot matmuls; normalize, +b2,
   log_softmax -> out [NPC, 2] f32.
"""
import sys

if "/opt/trn_rl_repo" not in sys.path:
    sys.path.insert(0, "/opt/trn_rl_repo")

import math
import numpy as np
import ml_dtypes

import concourse.bass as bass
import concourse.bacc as bacc
import concourse.mybir as mybir
import concourse.tile as tile
from concourse import bass_utils

P = 128
NEG = 0.2
NCHUNK = 4
NQUEUE = 4

# Tile's DMASW sem-lane assignment round-robins over all Pool DMAs, which
# breaks the per-lane FIFO assumption when SWDGE DMAs run on multiple queues
# (out-of-order completion across queues under one counting sem). Patch the
# lane choice to lane == queue_num: per-lane FIFO again holds (each HW ring
# drains in order), and queues get independent lanes.
from concourse import tile_sem_assignment as _tsa  # noqa: E402

if not getattr(_tsa.TileClockTick, "_qaware_patched", False):
    _orig_assign_tick = _tsa.TileClockTick._assign_tick

    def _qaware_assign_tick(self, inst):
        if (isinstance(inst, _tsa.DMAInst)
                and inst.engine == mybir.EngineType.Pool):
            self.next_sw_dma_idx = getattr(inst, "queue_num", 0) or 0
        return _orig_assign_tick(self, inst)

    _tsa.TileClockTick._assign_tick = _qaware_assign_tick
    _tsa.TileClockTick._qaware_patched = True


def _wrap16(flat):
    """[n] -> [128, n//16] wrapped in 16 partitions, replicated x8."""
    w = flat.reshape(-1, 16).T
    return np.tile(w, (8, 1))


# ----------------------------------------------------------------------------
# host-side data prep
# ----------------------------------------------------------------------------

def prep(inputs, cfg):
    N, F, H, C, CLS, NC = cfg["N"], cfg["F"], cfg["H"], cfg["C"], cfg["CLS"], cfg["NC"]
    SBG = cfg.get("SBG", 4)
    x = np.asarray(inputs["x"], np.float32)
    ei = np.asarray(inputs["edge_index"])
    W1 = np.asarray(inputs["W1"], np.float32)
    as1 = np.asarray(inputs["att_src1"], np.float32)
    ad1 = np.asarray(inputs["att_dst1"], np.float32)
    b1 = np.asarray(inputs["b1"], np.float32)
    W2 = np.asarray(inputs["W2"], np.float32)
    as2 = np.asarray(inputs["att_src2"], np.float32)
    ad2 = np.asarray(inputs["att_dst2"], np.float32)
    b2 = np.asarray(inputs["b2"], np.float32)

    HC = H * C
    R1 = HC + 2 * H                      # live row payload [h | asrc | adst]
    RG = 128 * math.ceil(R1 / 128)       # htab gather row elems (bf16, 256B mult)
    NPC = N // NC
    NB = math.ceil(NPC / P)
    NPCp = NB * P                        # padded local rows
    NT = (N + P - 1) // P
    Np = NT * P
    CHB = math.ceil(N / NCHUNK)          # chunk base (same partition L1 & L2)
    assert CHB + (Np - (NCHUNK - 1) * CHB) - CHB < 32768  # last-chunk slice
    assert CHB < 32768 and NPCp < 32768

    # ---- weights / constants -------------------------------------------------
    W1r = W1.reshape(F, H, C)
    Wsrc = np.einsum("fhc,hc->fh", W1r, as1)
    Wdst = np.einsum("fhc,hc->fh", W1r, ad1)
    W1aug = np.concatenate([W1, Wsrc, Wdst], axis=1)          # [F, R1]
    Wsrc2 = W2 @ as2.reshape(CLS, 1)
    Wdst2 = W2 @ ad2.reshape(CLS, 1)
    W2aug = np.concatenate([W2, Wsrc2, Wdst2], axis=1)        # [HC, 4]

    bf16 = ml_dtypes.bfloat16
    xT = np.zeros((F, Np), dtype=bf16)
    xT[:, :N] = x.T.astype(bf16)
    W1aug_b = W1aug.astype(bf16)
    W2aug_b = W2aug.astype(bf16)
    b1rep = np.tile(b1[None, :], (P, 1)).astype(bf16)
    b2rep = np.tile(b2[None, :], (P, 1)).astype(np.float32)
    iota = np.tile(np.arange(P, dtype=np.float32)[None, :], (P, 1)).astype(bf16)
    ident = np.eye(P, dtype=bf16)

    # ---- edges ---------------------------------------------------------------
    src_all = np.concatenate([ei[0], np.arange(N, dtype=ei.dtype)]).astype(np.int64)
    dst_all = np.concatenate([ei[1], np.arange(N, dtype=ei.dtype)]).astype(np.int64)
    order = np.argsort(dst_all, kind="stable")
    src_s = src_all[order]
    dst_s = dst_all[order]
    chunk_s = src_s // CHB

    cnts = np.zeros((NC, NB, NCHUNK), np.int64)
    for c in range(NC):
        for b in range(NB):
            base = c * NPC + b * P
            hi = min(base + P, (c + 1) * NPC)
            lo_i = np.searchsorted(dst_s, base)
            hi_i = np.searchsorted(dst_s, hi)
            ch = chunk_s[lo_i:hi_i]
            for q in range(NCHUNK):
                cnts[c, b, q] = (ch == q).sum()
    Trun = np.ceil(cnts / P).astype(np.int64).max(axis=0)     # [NB, NCHUNK]
    # ensure every block has >= 1 tile total (always true: self loops)

    # superblocks
    sblocks = [list(range(i, min(i + SBG, NB))) for i in range(0, NB, SBG)]
    # slot layout: per sb: for q: for b in sb: Trun[b,q] tiles
    sb_meta = []
    tile_base = 0
    for blist in sblocks:
        segs = []           # per q: (seg_tile_base_global, segT)
        runs = {b: [] for b in blist}   # block -> [(tile_global, T)]
        sb_base = tile_base
        for q in range(NCHUNK):
            segT = int(Trun[blist, q].sum())
            segs.append((tile_base, segT))
            tb = tile_base
            for b in blist:
                t = int(Trun[b, q])
                if t:
                    runs[b].append((tb, t))
                tb += t
            tile_base += segT
        sb_meta.append(dict(base=sb_base, S=tile_base - sb_base, segs=segs,
                            blocks=blist, runs=runs))
    Tsum = tile_base

    # per-core slot-value arrays
    ihsrc_w = np.zeros((NC, P, Tsum * 8), np.int16)
    dloc2d = np.full((NC, P, Tsum), 255.0, np.float32)
    dlocT_a = np.full((NC, 1, Tsum * P), 255.0, np.float32)
    for c in range(NC):
        ihsrc = np.zeros(Tsum * P, np.int16)
        dloc = np.full(Tsum * P, 255.0, np.float32)
        core_lo = np.searchsorted(dst_s, c * NPC)
        core_hi = np.searchsorted(dst_s, (c + 1) * NPC)
        cs, cd, cq = (src_s[core_lo:core_hi], dst_s[core_lo:core_hi],
                      chunk_s[core_lo:core_hi])
        # edges sorted by (dst, chunk); regroup per (block, chunk)
        for sb in sb_meta:
            for q in range(NCHUNK):
                tb = None
                for b in sb["blocks"]:
                    t = int(Trun[b, q])
                    if t == 0:
                        continue
                    # this block+chunk's edges (mask within the dst range)
                    base = c * NPC + b * P
                    hi = min(base + P, (c + 1) * NPC)
                    seg = slice(np.searchsorted(cd, base), np.searchsorted(cd, hi))
                    m = cq[seg] == q
                    es, ed = cs[seg][m], cd[seg][m]
                    n = len(es)
                    assert n <= t * P
                    # locate this run's global tile index (runs are in q order)
                    tg = None
                    for (tgi, tti) in sb["runs"][b]:
                        s0, sT = sb["segs"][q]
                        if s0 <= tgi < s0 + sT:
                            tg = tgi
                            break
                    assert tg is not None
                    s0 = tg * P
                    ihsrc[s0:s0 + n] = (es - q * CHB).astype(np.int16)
                    dloc[s0:s0 + n] = (ed - (c * NPC + b * P)).astype(np.float32)
        ihsrc_w[c] = _wrap16(ihsrc)
        dloc2d[c] = dloc.reshape(Tsum, P).T
        dlocT_a[c, 0] = dloc

    shared = {
        "xT": xT, "W1aug": W1aug_b, "W2aug": W2aug_b, "b1rep": b1rep,
        "b2rep": b2rep, "iota": iota, "ident": ident,
        "iotac": np.arange(P, dtype=np.float32).reshape(P, 1),
        "onesk": np.ones((1, P), np.float32),
    }
    in_maps = []
    for c in range(NC):
        m = dict(shared)
        xl = np.zeros((F, NPCp), dtype=bf16)
        xl[:, :NPC] = xT[:, c * NPC:c * NPC + NPC]
        m["xTloc"] = xl
        m["ihsrc"] = ihsrc_w[c]
        m["dloc2d"] = dloc2d[c]
        m["dlocT"] = dlocT_a[c]
        in_maps.append(m)

    meta = dict(cfg, R1=R1, RG=RG, HC=HC, NPC=NPC, NPCp=NPCp, NB=NB, NT=NT,
                Np=Np, CHB=CHB, Tsum=Tsum, sb_meta=sb_meta, SBG=SBG)
    return in_maps, meta


# ----------------------------------------------------------------------------
# device program
# ----------------------------------------------------------------------------

def _sub(ap, elem_off, dims):
    return bass.AP(ap.tensor, ap.offset + elem_off, [ap.ap[0], *list(dims)])


def build(meta, nc=None):
    N, F, H, C, CLS = meta["N"], meta["F"], meta["H"], meta["C"], meta["CLS"]
    NC, R1, RG, HC = meta["NC"], meta["R1"], meta["RG"], meta["HC"]
    NPC, NPCp, NB, NT, Np = (meta["NPC"], meta["NPCp"], meta["NB"], meta["NT"],
                             meta["Np"])
    CHB, Tsum = meta["CHB"], meta["Tsum"]
    sb_meta = meta["sb_meta"]
    R2 = CLS + 2
    RL2 = 64                           # f32 row elems for L2 gather tables

    f32, bf16, i16 = mybir.dt.float32, mybir.dt.bfloat16, mybir.dt.int16

    if nc is None:
        nc = bacc.Bacc("TRN2", target_bir_lowering=False, debug=False,
                       num_devices=NC, num_swdge_queues=NQUEUE)

    MAXT = 6                 # tiles per dma_gather call (<=768 descs, carveout 1024)
    qrr = [0]

    def gather_split(out_tile, rel, segT, elem, table, ix_tile):
        """Split a segment gather into <=MAXT-tile calls, round-robin queues."""
        done = 0
        while done < segT:
            tt = min(MAXT, segT - done)
            r = rel + done
            nc.gpsimd.dma_gather(
                bass.AP(out_tile[:].tensor, out_tile[:].offset + r * elem,
                        [out_tile[:].ap[0], [elem, tt], [1, elem]]),
                table,
                ix_tile[:, r * 8:(r + tt) * 8],
                tt * P, tt * P, elem,
                queue_num=qrr[0] % NQUEUE,
            )
            qrr[0] += 1
            done += tt

    xT_d = nc.dram_tensor("xT", [F, Np], bf16, kind="ExternalInput")
    xTl_d = nc.dram_tensor("xTloc", [F, NPCp], bf16, kind="ExternalInput")
    W1aug_d = nc.dram_tensor("W1aug", [F, R1], bf16, kind="ExternalInput")
    W2aug_d = nc.dram_tensor("W2aug", [HC, R2], bf16, kind="ExternalInput")
    b1rep_d = nc.dram_tensor("b1rep", [P, HC], bf16, kind="ExternalInput")
    b2rep_d = nc.dram_tensor("b2rep", [P, CLS], f32, kind="ExternalInput")
    iota_d = nc.dram_tensor("iota", [P, P], bf16, kind="ExternalInput")
    ident_d = nc.dram_tensor("ident", [P, P], bf16, kind="ExternalInput")
    ihsrc_d = nc.dram_tensor("ihsrc", [P, Tsum * 8], i16, kind="ExternalInput")
    dloc_d = nc.dram_tensor("dloc2d", [P, Tsum], f32, kind="ExternalInput")
    dlocT_d = nc.dram_tensor("dlocT", [1, Tsum * P], f32, kind="ExternalInput")
    iotac_d = nc.dram_tensor("iotac", [P, 1], f32, kind="ExternalInput")
    onesk_d = nc.dram_tensor("onesk", [1, P], f32, kind="ExternalInput")
    out_d = nc.dram_tensor("out", [NPC, CLS], f32, kind="ExternalOutput")

    htab = nc.dram_tensor("htab", [Np, RG], bf16, kind="Internal")
    sloc = nc.dram_tensor("sloc", [NPCp, H], bf16, kind="Internal")
    h2loc = nc.dram_tensor("h2loc", [NPC, R2], f32, kind="Internal")
    h2pad = nc.dram_tensor("h2pad", [NPCp, R2], f32, kind="Internal")
    h2tab = nc.dram_tensor("h2tab", [N, R2], f32, kind="Internal",
                           addr_space="Shared" if NC > 4 else "Local")
    h2tabp = nc.dram_tensor("h2tabp", [N, RL2], f32, kind="Internal")

    FA = min(P, F)
    FB = F - FA
    NCK = (HC + P - 1) // P

    with tile.TileContext(nc) as tc:
        with tc.tile_pool(name="const", bufs=1) as cp:
            w1a = cp.tile([FA, R1], bf16)
            nc.sync.dma_start(out=w1a[:], in_=W1aug_d[0:FA, :])
            if FB:
                w1b = cp.tile([FB, R1], bf16)
                nc.sync.dma_start(out=w1b[:], in_=W1aug_d[FA:F, :])
            w2s = []
            for k in range(NCK):
                kk = min(P, HC - k * P)
                w2k = cp.tile([kk, R2], bf16, name=f"w2k{k}")
                nc.sync.dma_start(out=w2k[:], in_=W2aug_d[k * P:k * P + kk, :])
                w2s.append(w2k)
            b1s = cp.tile([P, HC], bf16)
            nc.sync.dma_start(out=b1s[:], in_=b1rep_d[:, :])
            b2s = cp.tile([P, CLS], f32)
            nc.sync.dma_start(out=b2s[:], in_=b2rep_d[:, :])
            iot = cp.tile([P, P], bf16)
            nc.sync.dma_start(out=iot[:], in_=iota_d[:, :])
            idn = cp.tile([P, P], bf16)
            nc.sync.dma_start(out=idn[:], in_=ident_d[:, :])
            dlc = cp.tile([P, Tsum], f32)
            nc.sync.dma_start(out=dlc[:], in_=dloc_d[:, :])
            iotc = cp.tile([P, 1], f32)
            nc.sync.dma_start(out=iotc[:], in_=iotac_d[:, :])
            onek = cp.tile([1, P], f32)
            nc.sync.dma_start(out=onek[:], in_=onesk_d[:, :])

            # ---------------- Phase A: feature tables ------------------------
            with tc.tile_pool(name="pa", bufs=3) as pa, \
                 tc.tile_pool(name="psa", bufs=4, space="PSUM") as psa:
                for nt in range(NT):
                    xa = pa.tile([FA, P], bf16)
                    nc.sync.dma_start(out=xa[:], in_=xT_d[0:FA, nt * P:(nt + 1) * P])
                    if FB:
                        xb = pa.tile([FB, P], bf16)
                        nc.sync.dma_start(out=xb[:], in_=xT_d[FA:F, nt * P:(nt + 1) * P])
                    ph = psa.tile([P, R1], f32)
                    nc.tensor.matmul(out=ph[:], lhsT=xa[:], rhs=w1a[:],
                                     start=True, stop=(FB == 0))
                    if FB:
                        nc.tensor.matmul(out=ph[:], lhsT=xb[:], rhs=w1b[:],
                                         start=False, stop=True)
                    hsb = pa.tile([P, R1], bf16)
                    nc.vector.tensor_copy(out=hsb[:], in_=ph[:])
                    nc.sync.dma_start(
                        out=bass.AP(htab, nt * P * RG, [[RG, P], [1, R1]]),
                        in_=hsb[:])
                # local a_dst stats (for the dst-side gather)
                for nt in range(NPCp // P):
                    xa = pa.tile([FA, P], bf16, tag="xla")
                    nc.sync.dma_start(out=xa[:], in_=xTl_d[0:FA, nt * P:(nt + 1) * P])
                    if FB:
                        xb = pa.tile([FB, P], bf16, tag="xlb")
                        nc.sync.dma_start(out=xb[:], in_=xTl_d[FA:F, nt * P:(nt + 1) * P])
                    ps = psa.tile([P, H], f32, tag="pss")
                    nc.tensor.matmul(out=ps[:], lhsT=xa[:],
                                     rhs=w1a[:, HC + H:HC + 2 * H],
                                     start=True, stop=(FB == 0))
                    if FB:
                        nc.tensor.matmul(out=ps[:], lhsT=xb[:],
                                         rhs=w1b[:, HC + H:HC + 2 * H],
                                         start=False, stop=True)
                    ssb = pa.tile([P, H], bf16, tag="ssb")
                    nc.vector.tensor_copy(out=ssb[:], in_=ps[:])
                    nc.sync.dma_start(out=sloc[nt * P:(nt + 1) * P, :], in_=ssb[:])

            # ---------------- Phase B: L1 edge pass --------------------------
            with tc.tile_pool(name="pbg", bufs=2) as pbg, \
                 tc.tile_pool(name="pbb", bufs=2) as pbb, \
                 tc.tile_pool(name="psb", bufs=2, space="PSUM") as psb, \
                 tc.tile_pool(name="pst", bufs=1, space="PSUM") as pst, \
                 tc.tile_pool(name="psh", bufs=1, space="PSUM") as psh, \
                 tc.tile_pool(name="psk", bufs=2, space="PSUM") as psk, \
                 tc.tile_pool(name="psa2", bufs=2, space="PSUM") as psa2:
                for sb in sb_meta:
                    base, S = sb["base"], sb["S"]
                    nblk = len(sb["blocks"])
                    b0 = sb["blocks"][0]
                    g = pbg.tile([P, S * RG], bf16, tag="g")
                    ixs = pbg.tile([P, S * 8], i16, tag="ixs")
                    nc.sync.dma_start(out=ixs[:],
                                      in_=ihsrc_d[:, base * 8:(base + S) * 8])
                    for q in range(NCHUNK):
                        tb, segT = sb["segs"][q]
                        if segT == 0:
                            continue
                        hi = Np if q == NCHUNK - 1 else (q + 1) * CHB
                        gather_split(g, tb - base, segT, RG,
                                     htab[q * CHB:hi, :], ixs)
                    # a_dst window for the sb's blocks  [P, nblk*H] bf16
                    adw = pbg.tile([P, 8 * H], bf16, tag="adw")
                    nc.sync.dma_start(
                        out=adw[:, :nblk * H],
                        in_=bass.AP(sloc, b0 * P * H,
                                    [[H, P], [P * H, nblk], [1, H]]))
                    # O_T: [d, slot] one-hot via PE broadcast of dlocT + is_equal
                    dlT = pbg.tile([1, S * P], f32, tag="dlT")
                    nc.sync.dma_start(out=dlT[:],
                                      in_=dlocT_d[0:1, base * P:(base + S) * P])
                    oT = pbg.tile([P, S * P], bf16, tag="oT", bufs=1)
                    for st in range(0, S * P, 512):
                        w = min(512, S * P - st)
                        stp = psk.tile([P, 512], f32, tag="stp")
                        nc.tensor.matmul(out=stp[:, :w], lhsT=onek[:],
                                         rhs=dlT[0:1, st:st + w],
                                         start=True, stop=True)
                        nc.vector.tensor_tensor(
                            out=oT[:, st:st + w],
                            in0=iotc[:, 0:1].to_broadcast([P, w]),
                            in1=stp[:, :w],
                            op=mybir.AluOpType.is_equal)
                    # per-edge a_dst via O_T matmuls -> PSUM [P, S*H]
                    pad = psa2.tile([P, S * H], f32, tag="pad")
                    for bi, b in enumerate(sb["blocks"]):
                        for (tg, tt) in sb["runs"][b]:
                            for t in range(tt):
                                rel = tg - base + t
                                nc.tensor.matmul(
                                    out=pad[:, rel * H:(rel + 1) * H],
                                    lhsT=oT[:, rel * P:(rel + 1) * P],
                                    rhs=adw[:, bi * H:(bi + 1) * H],
                                    start=True, stop=True,
                                    skip_group_check=True)
                    # ex = exp(lrelu(asrc+adst)) for all slots  [P, S*H] f32
                    ex = pbb.tile([P, S * H], f32, tag="ex", bufs=1)
                    nc.vector.tensor_tensor(
                        out=ex[:].rearrange("p (t h) -> p t h", t=S),
                        in0=_sub(g[:], HC, [[RG, S], [1, H]]),
                        in1=pad[:].rearrange("p (t h) -> p t h", t=S),
                        op=mybir.AluOpType.add)
                    tmp = pbb.tile([P, S * H], f32, tag="tmp", bufs=1)
                    nc.vector.tensor_scalar_mul(out=tmp[:], in0=ex[:], scalar1=NEG)
                    nc.vector.tensor_tensor(out=ex[:], in0=ex[:], in1=tmp[:],
                                            op=mybir.AluOpType.max)
                    nc.scalar.activation(out=ex[:], in_=ex[:],
                                         func=mybir.ActivationFunctionType.Exp)
                    # msg in-place: cols 0:HC *= ex ; cols HC:HC+2H = ex
                    ex3 = ex[:].rearrange("p (t h) -> p t h", t=S)
                    nc.vector.tensor_tensor(
                        out=_sub(g[:], 0, [[RG, S], [C, H], [1, C]]),
                        in0=_sub(g[:], 0, [[RG, S], [C, H], [1, C]]),
                        in1=_sub(ex[:], 0, [[H, S], [1, H], [0, C]]),
                        op=mybir.AluOpType.mult)
                    nc.vector.tensor_copy(out=_sub(g[:], HC, [[RG, S], [1, H]]),
                                          in_=ex3)
                    nc.vector.tensor_copy(out=_sub(g[:], HC + H, [[RG, S], [1, H]]),
                                          in_=ex3)
                    # one-hot for all slots  [P, S*P] bf16
                    oh = pbb.tile([P, S * P], bf16, tag="oh", bufs=1)
                    nc.vector.tensor_tensor(
                        out=oh[:].rearrange("p (t q) -> p t q", t=S),
                        in0=_sub(dlc[:], base, [[1, S], [0, P]]),
                        in1=_sub(iot[:], 0, [[0, S], [1, P]]),
                        op=mybir.AluOpType.is_equal)
                    # per-block accumulation + normalize + L2 prep
                    for b in sb["blocks"]:
                        runs = sb["runs"][b]
                        ntile = sum(t for _, t in runs)
                        pso = psb.tile([P, R1], f32, tag="pso")
                        ti = 0
                        for (tg, tt) in runs:
                            for t in range(tt):
                                rel = tg - base + t
                                nc.tensor.matmul(
                                    out=pso[:],
                                    lhsT=oh[:, rel * P:(rel + 1) * P],
                                    rhs=g[:, rel * RG:rel * RG + R1],
                                    start=(ti == 0), stop=(ti == ntile - 1))
                                ti += 1
                        rows = min(P, NPC - b * P)
                        den = pbb.tile([P, H], f32, tag="den")
                        nc.vector.tensor_scalar_max(out=den[:],
                                                    in0=pso[:, HC:HC + H],
                                                    scalar1=1e-20)
                        rde = pbb.tile([P, H], f32, tag="rde")
                        nc.vector.reciprocal(out=rde[:], in_=den[:])
                        o1 = pbb.tile([P, HC], bf16, tag="o1")
                        for hh in range(H):
                            nc.vector.tensor_scalar_mul(
                                out=o1[:, hh * C:(hh + 1) * C],
                                in0=pso[:, hh * C:(hh + 1) * C],
                                scalar1=rde[:, hh:hh + 1])
                        nc.vector.tensor_tensor(out=o1[:], in0=o1[:], in1=b1s[:],
                                                op=mybir.AluOpType.add)
                        nc.scalar.activation(out=o1[:], in_=o1[:],
                                             func=mybir.ActivationFunctionType.Relu)
                        ph2 = psh.tile([P, R2], f32, tag="ph2")
                        for k in range(NCK):
                            kk = min(P, HC - k * P)
                            ptr = pst.tile([P, P], bf16, tag="ptr")
                            nc.tensor.transpose(out=ptr[:kk, :],
                                                in_=o1[:, k * P:k * P + kk],
                                                identity=idn[:])
                            rT = pbb.tile([P, P], bf16, tag="rT")
                            nc.vector.tensor_copy(out=rT[:kk, :], in_=ptr[:kk, :])
                            nc.tensor.matmul(out=ph2[:], lhsT=rT[:kk, :],
                                             rhs=w2s[k][:kk, :],
                                             start=(k == 0), stop=(k == NCK - 1))
                        h2s = pbb.tile([P, R2], f32, tag="h2s")
                        nc.vector.tensor_copy(out=h2s[:], in_=ph2[:])
                        nc.sync.dma_start(out=h2loc[b * P:b * P + rows, :],
                                          in_=h2s[:rows, :])
                        nc.sync.dma_start(out=h2pad[b * P:(b + 1) * P, :],
                                          in_=h2s[:])

            # ---------------- AllGather + repack -----------------------------
            nc.gpsimd.collective_compute(
                "AllGather", mybir.AluOpType.bypass,
                replica_groups=[list(range(NC))],
                ins=[h2loc[:, :]], outs=[h2tab[:, :]])
            # repack [N, R2] -> 256B rows [N, RL2]
            for r in range(NC):
                nc.sync.dma_start(
                    out=bass.AP(h2tabp, r * NPC * RL2, [[RL2, NPC], [1, R2]]),
                    in_=h2tab[r * NPC:(r + 1) * NPC, :])

            # ---------------- Phase C: L2 edge pass --------------------------
            with tc.tile_pool(name="pcg", bufs=2) as pcg, \
                 tc.tile_pool(name="pcb", bufs=2) as pcb, \
                 tc.tile_pool(name="psc", bufs=2, space="PSUM") as psc, \
                 tc.tile_pool(name="psk2", bufs=2, space="PSUM") as psk2, \
                 tc.tile_pool(name="psd2", bufs=2, space="PSUM") as psd2:
                for sb in sb_meta:
                    base, S = sb["base"], sb["S"]
                    nblk = len(sb["blocks"])
                    b0 = sb["blocks"][0]
                    g2 = pcg.tile([P, S * RL2], f32, tag="g2")
                    ixs = pcg.tile([P, S * 8], i16, tag="ixs2")
                    nc.sync.dma_start(out=ixs[:],
                                      in_=ihsrc_d[:, base * 8:(base + S) * 8])
                    for q in range(NCHUNK):
                        tb, segT = sb["segs"][q]
                        if segT == 0:
                            continue
                        hi = N if q == NCHUNK - 1 else (q + 1) * CHB
                        gather_split(g2, tb - base, segT, RL2,
                                     h2tabp[q * CHB:hi, :], ixs)
                    adw2 = pcg.tile([P, 8], bf16, tag="adw2")
                    nc.gpsimd.dma_start(
                        out=adw2[:, :nblk],
                        in_=bass.AP(h2pad, b0 * P * R2 + CLS + 1,
                                    [[R2, P], [P * R2, nblk], [1, 1]]))
                    dlT = pcg.tile([1, S * P], f32, tag="dlT2")
                    nc.sync.dma_start(out=dlT[:],
                                      in_=dlocT_d[0:1, base * P:(base + S) * P])
                    oT = pcg.tile([P, S * P], bf16, tag="oT2", bufs=1)
                    for st in range(0, S * P, 512):
                        w = min(512, S * P - st)
                        stp = psk2.tile([P, 512], f32, tag="stp2")
                        nc.tensor.matmul(out=stp[:, :w], lhsT=onek[:],
                                         rhs=dlT[0:1, st:st + w],
                                         start=True, stop=True)
                        nc.vector.tensor_tensor(
                            out=oT[:, st:st + w],
                            in0=iotc[:, 0:1].to_broadcast([P, w]),
                            in1=stp[:, :w],
                            op=mybir.AluOpType.is_equal)
                    pad2 = psd2.tile([P, S], f32, tag="pad2")
                    for bi, b in enumerate(sb["blocks"]):
                        for (tg, tt) in sb["runs"][b]:
                            for t in range(tt):
                                rel = tg - base + t
                                nc.tensor.matmul(
                                    out=pad2[:, rel:rel + 1],
                                    lhsT=oT[:, rel * P:(rel + 1) * P],
                                    rhs=adw2[:, bi:bi + 1],
                                    start=True, stop=True,
                                    skip_group_check=True)
                    ex2 = pcb.tile([P, S], f32, tag="ex2")
                    nc.vector.tensor_tensor(
                        out=ex2[:],
                        in0=_sub(g2[:], CLS, [[RL2, S]]),
                        in1=pad2[:],
                        op=mybir.AluOpType.add)
                    tm2 = pcb.tile([P, S], f32, tag="tm2")
                    nc.vector.tensor_scalar_mul(out=tm2[:], in0=ex2[:], scalar1=NEG)
                    nc.vector.tensor_tensor(out=ex2[:], in0=ex2[:], in1=tm2[:],
                                            op=mybir.AluOpType.max)
                    nc.scalar.activation(out=ex2[:], in_=ex2[:],
                                         func=mybir.ActivationFunctionType.Exp)
                    m2 = pcb.tile([P, S * 4], bf16, tag="m2")
                    nc.vector.tensor_copy(out=_sub(m2[:], CLS, [[4, S]]), in_=ex2[:])
                    nc.vector.tensor_copy(out=_sub(m2[:], CLS + 1, [[4, S]]),
                                          in_=ex2[:])
                    nc.vector.tensor_tensor(
                        out=_sub(m2[:], 0, [[4, S], [1, CLS]]),
                        in0=_sub(g2[:], 0, [[RL2, S], [1, CLS]]),
                        in1=_sub(m2[:], CLS, [[4, S], [0, CLS]]),
                        op=mybir.AluOpType.mult)
                    oh2 = pcb.tile([P, S * P], bf16, tag="oh2", bufs=1)
                    nc.vector.tensor_tensor(
                        out=oh2[:].rearrange("p (t q) -> p t q", t=S),
                        in0=_sub(dlc[:], base, [[1, S], [0, P]]),
                        in1=_sub(iot[:], 0, [[0, S], [1, P]]),
                        op=mybir.AluOpType.is_equal)
                    for b in sb["blocks"]:
                        runs = sb["runs"][b]
                        ntile = sum(t for _, t in runs)
                        ps2 = psc.tile([P, 4], f32, tag="ps2")
                        ti = 0
                        for (tg, tt) in runs:
                            for t in range(tt):
                                rel = tg - base + t
                                nc.tensor.matmul(
                                    out=ps2[:],
                                    lhsT=oh2[:, rel * P:(rel + 1) * P],
                                    rhs=m2[:, rel * 4:(rel + 1) * 4],
                                    start=(ti == 0), stop=(ti == ntile - 1))
                                ti += 1
                        rows = min(P, NPC - b * P)
                        den2 = pcb.tile([P, 1], f32, tag="den2")
                        nc.vector.tensor_scalar_max(out=den2[:],
                                                    in0=ps2[:, CLS:CLS + 1],
                                                    scalar1=1e-20)
                        rd2 = pcb.tile([P, 1], f32, tag="rd2")
                        nc.vector.reciprocal(out=rd2[:], in_=den2[:])
                        v = pcb.tile([P, CLS], f32, tag="v")
                        nc.vector.tensor_scalar_mul(out=v[:], in0=ps2[:, 0:CLS],
                                                    scalar1=rd2[:, 0:1])
                        nc.vector.tensor_tensor(out=v[:], in0=v[:], in1=b2s[:],
                                                op=mybir.AluOpType.add)
                        mx = pcb.tile([P, 1], f32, tag="mx")
                        nc.vector.tensor_reduce(out=mx[:], in_=v[:],
                                                axis=mybir.AxisListType.X,
                                                op=mybir.AluOpType.max)
                        u = pcb.tile([P, CLS], f32, tag="u")
                        nc.vector.tensor_scalar_sub(out=u[:], in0=v[:],
                                                    scalar1=mx[:, 0:1])
                        nc.scalar.activation(out=u[:], in_=u[:],
                                             func=mybir.ActivationFunctionType.Exp)
                        sm = pcb.tile([P, 1], f32, tag="sm")
                        nc.vector.tensor_reduce(out=sm[:], in_=u[:],
                                                axis=mybir.AxisListType.X,
                                                op=mybir.AluOpType.add)
                        ls = pcb.tile([P, 1], f32, tag="ls")
                        nc.scalar.activation(out=ls[:], in_=sm[:],
                                             func=mybir.ActivationFunctionType.Ln)
                        nc.vector.tensor_tensor(out=ls[:], in0=ls[:], in1=mx[:],
                                                op=mybir.AluOpType.add)
                        res = pcb.tile([P, CLS], f32, tag="res")
                        nc.vector.tensor_scalar_sub(out=res[:], in0=v[:],
                                                    scalar1=ls[:, 0:1])
                        nc.sync.dma_start(out=out_d[b * P:b * P + rows, :],
                                          in_=res[:rows, :])
    nc.finalize()
    return nc


def install_ntff_hook(so_path="/opt/axon/libaxon_pjrt.so"):
    import types
    import ctypes
    import contextlib
    import antenv

    if getattr(antenv, "axon_hooks", None) is not None:
        return
    lib = ctypes.CDLL(so_path)
    if not hasattr(lib, "axon_start_nrt_profile"):
        return
    lib.axon_start_nrt_profile.argtypes = [ctypes.POINTER(ctypes.c_int64),
                                           ctypes.c_size_t]
    lib.axon_start_nrt_profile.restype = ctypes.c_int64
    lib.axon_stop_nrt_profile.argtypes = [ctypes.c_char_p]
    lib.axon_stop_nrt_profile.restype = ctypes.c_int64

    @contextlib.contextmanager
    def _hook(output_dir, device_ids):
        import jax
        jax.devices()
        if device_ids:
            ids = (ctypes.c_int64 * len(device_ids))(*device_ids)
            rc = lib.axon_start_nrt_profile(ids, len(device_ids))
        else:
            rc = lib.axon_start_nrt_profile(None, 0)
        if rc != 0:
            raise RuntimeError(f"axon_start_nrt_profile rc={rc}")
        try:
            yield
        finally:
            n = lib.axon_stop_nrt_profile(str(output_dir).encode())
            print(f"ntff profile: {n} file(s) written to {output_dir}")

    mod = types.ModuleType("antenv.axon_hooks")
    _reg = [_hook]
    mod.set_axon_ntff_profile_hook = lambda h: _reg.__setitem__(0, h)
    mod.get_axon_ntff_profile_hook = lambda: _reg[0]
    sys.modules["antenv.axon_hooks"] = mod
    antenv.axon_hooks = mod


def run(inputs, cfg, trace=False, **kwargs):
    if trace:
        install_ntff_hook()
    in_maps, meta = prep(inputs, cfg)
    nc = build(meta)
    res = bass_utils.run_bass_kernel_spmd(
        nc, in_maps, core_ids=list(range(cfg["NC"])), trace=trace, **kwargs)
    out = np.concatenate([res.results[c]["out"] for c in range(cfg["NC"])], axis=0)
    return out, res


# ----------------------------------------------------------------------------
# harness entry point
# ----------------------------------------------------------------------------

_CFG = dict(N=100000, F=165, H=4, C=64, CLS=2, NC=8)


def kernel(**inputs):
    """Full (unsharded) inputs -> full [N, 2] float32 log-softmax output.

    Shards edges by destination-node range across the 8 NeuronCores,
    compiles and runs the Bass/Tile kernel via run_bass_kernel_spmd,
    and concatenates the per-core output slices.
    """
    out, _ = run(inputs, _CFG, trace=False)
    return np.ascontiguousarray(out.astype(np.float32))



# revision 5
# speedup vs baseline: 1.5124x; 1.5124x over previous
"""GAT 2-layer message-passing network on 8 TRN2 NeuronCores (Bass/Tile).

v3: batched phase A (4 node-blocks per DMA group), chunk-split htab tensors
(CHB=25088, 128*4-block aligned) for A/B overlap, bf16 one-hot tables,
single ex column (rhs N=260), DVE relu, batched final log-softmax via
Softplus, per-sb h2 write batching, MAXT=8 gather calls.

Strategy (dst-sharded):
 - Host: add self loops, sort edges by dst, shard dst-node ranges across cores.
   Each core owns nodes [c*NPC, (c+1)*NPC) and ALL edges into them.
 - Edge slots: per dst-block of 128 nodes, edges sub-grouped by src chunk
   (4 chunks of CHB rows so int16 indices work), each (block,chunk) run padded
   to x128 slots = tiles. Superblocks of SBG blocks share gather calls.
 - Phase A (replicated): full feature table htab[n] = [h|a_src|a_dst|pad] bf16
   rows (768B) split into 4 per-chunk DRAM tensors, + local stats sloc
   [NPCp, H] for the core's own nodes (from xT_loc).
 - Phase B (L1): per sb: dma_gather htab rows by src (4 chunk calls);
   per-edge a_dst via one-hot-transpose matmuls; ex = exp(lrelu(asrc+adst));
   msg in-place in gather buffer ([h*ex|ex] in cols 0:260); one-hot
   accumulation matmuls per dst block; normalize, +b1, relu; h2aug = relu @
   W2aug via PE transpose; write h2loc (padded rows, one DMA per sb).
 - AllGather h2loc[:NPC] -> h2tab [N,4] f32; repack into h2tabp [Np, 64] f32.
 - Phase C (L2): same slots: gather h2tabp rows by src; same one-hot scheme;
   per-block v = num/den collected in SBUF; final batched log-softmax via
   Softplus; single strided output DMA.
"""
import sys

if "/opt/trn_rl_repo" not in sys.path:
    sys.path.insert(0, "/opt/trn_rl_repo")

import math
import numpy as np
import ml_dtypes

import concourse.bass as bass
import concourse.bacc as bacc
import concourse.mybir as mybir
import concourse.tile as tile
from concourse import bass_utils

P = 128
NEG = 0.2
NCHUNK = 4
NQUEUE = 4
MAXT = 8                 # tiles per dma_gather call

# Tile's DMASW sem-lane assignment round-robins over all Pool DMAs, which
# breaks the per-lane FIFO assumption when SWDGE DMAs run on multiple queues
# (out-of-order completion across queues under one counting sem). Patch the
# lane choice to lane == queue_num: per-lane FIFO again holds (each HW ring
# drains in order), and queues get independent lanes.
from concourse import tile_sem_assignment as _tsa  # noqa: E402

if not getattr(_tsa.TileClockTick, "_qaware_patched", False):
    _orig_assign_tick = _tsa.TileClockTick._assign_tick

    def _qaware_assign_tick(self, inst):
        if (isinstance(inst, _tsa.DMAInst)
                and inst.engine == mybir.EngineType.Pool):
            self.next_sw_dma_idx = getattr(inst, "queue_num", 0) or 0
        return _orig_assign_tick(self, inst)

    _tsa.TileClockTick._assign_tick = _qaware_assign_tick
    _tsa.TileClockTick._qaware_patched = True


def _wrap16(flat):
    """[n] -> [128, n//16] wrapped in 16 partitions, replicated x8."""
    w = flat.reshape(-1, 16).T
    return np.tile(w, (8, 1))


# ----------------------------------------------------------------------------
# host-side data prep
# ----------------------------------------------------------------------------

def prep(inputs, cfg):
    N, F, H, C, CLS, NC = cfg["N"], cfg["F"], cfg["H"], cfg["C"], cfg["CLS"], cfg["NC"]
    SBG = cfg.get("SBG", 4)
    x = np.asarray(inputs["x"], np.float32)
    ei = np.asarray(inputs["edge_index"])
    W1 = np.asarray(inputs["W1"], np.float32)
    as1 = np.asarray(inputs["att_src1"], np.float32)
    ad1 = np.asarray(inputs["att_dst1"], np.float32)
    b1 = np.asarray(inputs["b1"], np.float32)
    W2 = np.asarray(inputs["W2"], np.float32)
    as2 = np.asarray(inputs["att_src2"], np.float32)
    ad2 = np.asarray(inputs["att_dst2"], np.float32)
    b2 = np.asarray(inputs["b2"], np.float32)

    HC = H * C
    R1 = HC + 2 * H                      # live row payload [h | asrc | adst]
    RG = 128 * math.ceil(R1 / 128)       # htab gather row elems (bf16, 256B mult)
    NPC = N // NC
    NB = math.ceil(NPC / P)
    NPCp = NB * P                        # padded local rows
    NT = (N + P - 1) // P
    Np = NT * P
    CHB = 25088                          # chunk rows (=196 blocks, 4-aligned)
    assert NT == 782 and 3 * CHB < Np
    assert CHB < 32768 and NPCp < 32768

    # ---- weights / constants -------------------------------------------------
    W1r = W1.reshape(F, H, C)
    Wsrc = np.einsum("fhc,hc->fh", W1r, as1)
    Wdst = np.einsum("fhc,hc->fh", W1r, ad1)
    W1aug = np.concatenate([W1, Wsrc, Wdst], axis=1)          # [F, R1]
    Wsrc2 = W2 @ as2.reshape(CLS, 1)
    Wdst2 = W2 @ ad2.reshape(CLS, 1)
    W2aug = np.concatenate([W2, Wsrc2, Wdst2], axis=1)        # [HC, 4]

    bf16 = ml_dtypes.bfloat16
    xT = np.zeros((F, Np), dtype=bf16)
    xT[:, :N] = x.T.astype(bf16)
    W1aug_b = W1aug.astype(bf16)
    W2aug_b = W2aug.astype(bf16)
    b1rep = np.tile(b1[None, :], (P, 1)).astype(bf16)
    b2all = np.tile(b2[None, :], (P, NB)).astype(np.float32)  # [P, NB*CLS]
    iota = np.tile(np.arange(P, dtype=np.float32)[None, :], (P, 1)).astype(bf16)
    ident = np.eye(P, dtype=bf16)

    # ---- edges ---------------------------------------------------------------
    src_all = np.concatenate([ei[0], np.arange(N, dtype=ei.dtype)]).astype(np.int64)
    dst_all = np.concatenate([ei[1], np.arange(N, dtype=ei.dtype)]).astype(np.int64)
    order = np.argsort(dst_all, kind="stable")
    src_s = src_all[order]
    dst_s = dst_all[order]
    chunk_s = src_s // CHB

    cnts = np.zeros((NC, NB, NCHUNK), np.int64)
    for c in range(NC):
        for b in range(NB):
            base = c * NPC + b * P
            hi = min(base + P, (c + 1) * NPC)
            lo_i = np.searchsorted(dst_s, base)
            hi_i = np.searchsorted(dst_s, hi)
            ch = chunk_s[lo_i:hi_i]
            for q in range(NCHUNK):
                cnts[c, b, q] = (ch == q).sum()
    Trun = np.ceil(cnts / P).astype(np.int64).max(axis=0)     # [NB, NCHUNK]

    # superblocks
    sblocks = [list(range(i, min(i + SBG, NB))) for i in range(0, NB, SBG)]
    # slot layout: per sb: for q: for b in sb: Trun[b,q] tiles
    sb_meta = []
    tile_base = 0
    for blist in sblocks:
        segs = []           # per q: (seg_tile_base_global, segT)
        runs = {b: [] for b in blist}   # block -> [(tile_global, T)]
        sb_base = tile_base
        for q in range(NCHUNK):
            segT = int(Trun[blist, q].sum())
            segs.append((tile_base, segT))
            tb = tile_base
            for b in blist:
                t = int(Trun[b, q])
                if t:
                    runs[b].append((tb, t))
                tb += t
            tile_base += segT
        sb_meta.append(dict(base=sb_base, S=tile_base - sb_base, segs=segs,
                            blocks=blist, runs=runs))
    Tsum = tile_base

    # per-core slot-value arrays
    ihsrc_w = np.zeros((NC, P, Tsum * 8), np.int16)
    dloc2d = np.full((NC, P, Tsum), 255.0, bf16)
    dlocT_a = np.full((NC, 1, Tsum * P), 255.0, bf16)
    for c in range(NC):
        ihsrc = np.zeros(Tsum * P, np.int16)
        dloc = np.full(Tsum * P, 255.0, np.float32)
        core_lo = np.searchsorted(dst_s, c * NPC)
        core_hi = np.searchsorted(dst_s, (c + 1) * NPC)
        cs, cd, cq = (src_s[core_lo:core_hi], dst_s[core_lo:core_hi],
                      chunk_s[core_lo:core_hi])
        # edges sorted by (dst, chunk); regroup per (block, chunk)
        for sb in sb_meta:
            for q in range(NCHUNK):
                for b in sb["blocks"]:
                    t = int(Trun[b, q])
                    if t == 0:
                        continue
                    base = c * NPC + b * P
                    hi = min(base + P, (c + 1) * NPC)
                    seg = slice(np.searchsorted(cd, base), np.searchsorted(cd, hi))
                    m = cq[seg] == q
                    es, ed = cs[seg][m], cd[seg][m]
                    n = len(es)
                    assert n <= t * P
                    tg = None
                    for (tgi, tti) in sb["runs"][b]:
                        s0, sT = sb["segs"][q]
                        if s0 <= tgi < s0 + sT:
                            tg = tgi
                            break
                    assert tg is not None
                    s0 = tg * P
                    ihsrc[s0:s0 + n] = (es - q * CHB).astype(np.int16)
                    dloc[s0:s0 + n] = (ed - (c * NPC + b * P)).astype(np.float32)
        ihsrc_w[c] = _wrap16(ihsrc)
        dloc2d[c] = dloc.reshape(Tsum, P).T.astype(bf16)
        dlocT_a[c, 0] = dloc.astype(bf16)

    shared = {
        "xT": xT, "W1aug": W1aug_b, "W2aug": W2aug_b, "b1rep": b1rep,
        "b2all": b2all, "iota": iota, "ident": ident,
        "iotac": np.arange(P, dtype=np.float32).reshape(P, 1),
        "onesk": np.ones((1, P), bf16),
    }
    in_maps = []
    for c in range(NC):
        m = dict(shared)
        xl = np.zeros((F, NPCp), dtype=bf16)
        xl[:, :NPC] = xT[:, c * NPC:c * NPC + NPC]
        m["xTloc"] = xl
        m["ihsrc"] = ihsrc_w[c]
        m["dloc2d"] = dloc2d[c]
        m["dlocT"] = dlocT_a[c]
        in_maps.append(m)

    meta = dict(cfg, R1=R1, RG=RG, HC=HC, NPC=NPC, NPCp=NPCp, NB=NB, NT=NT,
                Np=Np, CHB=CHB, Tsum=Tsum, sb_meta=sb_meta, SBG=SBG)
    return in_maps, meta


# ----------------------------------------------------------------------------
# device program
# ----------------------------------------------------------------------------

def _sub(ap, elem_off, dims):
    return bass.AP(ap.tensor, ap.offset + elem_off, [ap.ap[0], *list(dims)])


def build(meta, nc=None):
    N, F, H, C, CLS = meta["N"], meta["F"], meta["H"], meta["C"], meta["CLS"]
    NC, R1, RG, HC = meta["NC"], meta["R1"], meta["RG"], meta["HC"]
    NPC, NPCp, NB, NT, Np = (meta["NPC"], meta["NPCp"], meta["NB"], meta["NT"],
                             meta["Np"])
    CHB, Tsum = meta["CHB"], meta["Tsum"]
    sb_meta = meta["sb_meta"]
    R2 = CLS + 2
    RL2 = 64                           # f32 row elems for L2 gather tables
    RUSE = HC + H                      # accum-matmul rhs width [h*ex | ex]

    f32, bf16, i16 = mybir.dt.float32, mybir.dt.bfloat16, mybir.dt.int16

    if nc is None:
        nc = bacc.Bacc("TRN2", target_bir_lowering=False, debug=False,
                       num_devices=NC, num_swdge_queues=NQUEUE)

    qrr = [0]

    def gather_split(out_tile, rel, segT, elem, table, ix_tile):
        """Split a segment gather into <=MAXT-tile calls, round-robin queues."""
        done = 0
        while done < segT:
            tt = min(MAXT, segT - done)
            r = rel + done
            nc.gpsimd.dma_gather(
                bass.AP(out_tile[:].tensor, out_tile[:].offset + r * elem,
                        [out_tile[:].ap[0], [elem, tt], [1, elem]]),
                table,
                ix_tile[:, r * 8:(r + tt) * 8],
                tt * P, tt * P, elem,
                queue_num=qrr[0] % NQUEUE,
            )
            qrr[0] += 1
            done += tt

    xT_d = nc.dram_tensor("xT", [F, Np], bf16, kind="ExternalInput")
    xTl_d = nc.dram_tensor("xTloc", [F, NPCp], bf16, kind="ExternalInput")
    W1aug_d = nc.dram_tensor("W1aug", [F, R1], bf16, kind="ExternalInput")
    W2aug_d = nc.dram_tensor("W2aug", [HC, R2], bf16, kind="ExternalInput")
    b1rep_d = nc.dram_tensor("b1rep", [P, HC], bf16, kind="ExternalInput")
    b2all_d = nc.dram_tensor("b2all", [P, NB * CLS], f32, kind="ExternalInput")
    iota_d = nc.dram_tensor("iota", [P, P], bf16, kind="ExternalInput")
    ident_d = nc.dram_tensor("ident", [P, P], bf16, kind="ExternalInput")
    ihsrc_d = nc.dram_tensor("ihsrc", [P, Tsum * 8], i16, kind="ExternalInput")
    dloc_d = nc.dram_tensor("dloc2d", [P, Tsum], bf16, kind="ExternalInput")
    dlocT_d = nc.dram_tensor("dlocT", [1, Tsum * P], bf16, kind="ExternalInput")
    iotac_d = nc.dram_tensor("iotac", [P, 1], f32, kind="ExternalInput")
    onesk_d = nc.dram_tensor("onesk", [1, P], bf16, kind="ExternalInput")
    out_d = nc.dram_tensor("out", [NPC, CLS], f32, kind="ExternalOutput")

    # per-chunk feature tables (768B rows); last chunk holds the remainder
    CH_ROWS = [CHB, CHB, CHB, Np - 3 * CHB]
    htabs = [nc.dram_tensor(f"htab{q}", [CH_ROWS[q], RG], bf16, kind="Internal")
             for q in range(NCHUNK)]
    sloc = nc.dram_tensor("sloc", [NPCp, H], bf16, kind="Internal")
    h2loc = nc.dram_tensor("h2loc", [NPCp, R2], f32, kind="Internal")
    h2tab = nc.dram_tensor("h2tab", [N, R2], f32, kind="Internal",
                           addr_space="Shared" if NC > 4 else "Local")
    h2tabp = nc.dram_tensor("h2tabp", [N, RL2], f32, kind="Internal")

    FA = min(P, F)
    FB = F - FA
    NCK = (HC + P - 1) // P
    GRP = 4                            # node blocks per phase-A group
    CHBL = CHB // P                    # blocks per chunk (196)

    with tile.TileContext(nc) as tc:
        with tc.tile_pool(name="const", bufs=1) as cp:
            w1a = cp.tile([FA, R1], bf16)
            nc.sync.dma_start(out=w1a[:], in_=W1aug_d[0:FA, :])
            w1b = cp.tile([FB, R1], bf16)
            nc.sync.dma_start(out=w1b[:], in_=W1aug_d[FA:F, :])
            w2s = []
            for k in range(NCK):
                kk = min(P, HC - k * P)
                w2k = cp.tile([kk, R2], bf16, name=f"w2k{k}")
                nc.sync.dma_start(out=w2k[:], in_=W2aug_d[k * P:k * P + kk, :])
                w2s.append(w2k)
            b1s = cp.tile([P, HC], bf16)
            nc.sync.dma_start(out=b1s[:], in_=b1rep_d[:, :])
            b2a = cp.tile([P, NB * CLS], f32)
            nc.sync.dma_start(out=b2a[:], in_=b2all_d[:, :])
            iot = cp.tile([P, P], bf16)
            nc.sync.dma_start(out=iot[:], in_=iota_d[:, :])
            idn = cp.tile([P, P], bf16)
            nc.sync.dma_start(out=idn[:], in_=ident_d[:, :])
            dlc = cp.tile([P, Tsum], bf16)
            nc.sync.dma_start(out=dlc[:], in_=dloc_d[:, :])
            iotc = cp.tile([P, 1], f32)
            nc.sync.dma_start(out=iotc[:], in_=iotac_d[:, :])
            onek = cp.tile([1, P], bf16)
            nc.sync.dma_start(out=onek[:], in_=onesk_d[:, :])
            vall = cp.tile([P, NB * CLS], f32)   # phase-C per-node logits

            # ---------------- Phase A: feature tables ------------------------
            with tc.tile_pool(name="pa", bufs=3) as pa, \
                 tc.tile_pool(name="psa", bufs=4, space="PSUM") as psa:
                for q in range(NCHUNK):
                    nbq = CH_ROWS[q] // P
                    for g0 in range(0, nbq, GRP):
                        glen = min(GRP, nbq - g0)
                        col0 = q * CHB + g0 * P
                        w = glen * P
                        xa = pa.tile([FA, GRP * P], bf16, tag="xa")
                        nc.sync.dma_start(out=xa[:, :w],
                                          in_=xT_d[0:FA, col0:col0 + w])
                        xb = pa.tile([FB, GRP * P], bf16, tag="xb")
                        nc.sync.dma_start(out=xb[:, :w],
                                          in_=xT_d[FA:F, col0:col0 + w])
                        hsb = pa.tile([P, GRP * R1], bf16, tag="hsb")
                        for k in range(glen):
                            ph = psa.tile([P, R1], f32, tag="ph")
                            nc.tensor.matmul(out=ph[:], lhsT=xa[:, k * P:(k + 1) * P],
                                             rhs=w1a[:], start=True, stop=False)
                            nc.tensor.matmul(out=ph[:], lhsT=xb[:, k * P:(k + 1) * P],
                                             rhs=w1b[:], start=False, stop=True)
                            nc.vector.tensor_copy(out=hsb[:, k * R1:(k + 1) * R1],
                                                  in_=ph[:])
                        nc.sync.dma_start(
                            out=bass.AP(htabs[q], g0 * P * RG,
                                        [[RG, P], [P * RG, glen], [1, R1]]),
                            in_=hsb[:, :glen * R1].rearrange(
                                "p (g r) -> p g r", g=glen))
                # local a_dst stats (for the dst-side window loads)
                for g0 in range(0, NPCp // P, GRP):
                    glen = min(GRP, NPCp // P - g0)
                    w = glen * P
                    xa = pa.tile([FA, GRP * P], bf16, tag="xla")
                    nc.sync.dma_start(out=xa[:, :w],
                                      in_=xTl_d[0:FA, g0 * P:g0 * P + w])
                    xb = pa.tile([FB, GRP * P], bf16, tag="xlb")
                    nc.sync.dma_start(out=xb[:, :w],
                                      in_=xTl_d[FA:F, g0 * P:g0 * P + w])
                    ssb = pa.tile([P, GRP * H], bf16, tag="ssb")
                    for k in range(glen):
                        ps = psa.tile([P, H], f32, tag="pss")
                        nc.tensor.matmul(out=ps[:], lhsT=xa[:, k * P:(k + 1) * P],
                                         rhs=w1a[:, HC + H:HC + 2 * H],
                                         start=True, stop=False)
                        nc.tensor.matmul(out=ps[:], lhsT=xb[:, k * P:(k + 1) * P],
                                         rhs=w1b[:, HC + H:HC + 2 * H],
                                         start=False, stop=True)
                        nc.vector.tensor_copy(out=ssb[:, k * H:(k + 1) * H],
                                              in_=ps[:])
                    nc.sync.dma_start(
                        out=bass.AP(sloc, g0 * P * H,
                                    [[H, P], [P * H, glen], [1, H]]),
                        in_=ssb[:, :glen * H].rearrange(
                            "p (g r) -> p g r", g=glen))

            # ---------------- Phase B: L1 edge pass --------------------------
            with tc.tile_pool(name="pbg", bufs=2) as pbg, \
                 tc.tile_pool(name="pbb", bufs=2) as pbb, \
                 tc.tile_pool(name="psb", bufs=2, space="PSUM") as psb, \
                 tc.tile_pool(name="pst", bufs=1, space="PSUM") as pst, \
                 tc.tile_pool(name="psh", bufs=1, space="PSUM") as psh, \
                 tc.tile_pool(name="psk", bufs=2, space="PSUM") as psk, \
                 tc.tile_pool(name="psa2", bufs=2, space="PSUM") as psa2:
                for sb in sb_meta:
                    base, S = sb["base"], sb["S"]
                    nblk = len(sb["blocks"])
                    b0 = sb["blocks"][0]
                    g = pbg.tile([P, S * RG], bf16, tag="g")
                    ixs = pbg.tile([P, S * 8], i16, tag="ixs")
                    nc.sync.dma_start(out=ixs[:],
                                      in_=ihsrc_d[:, base * 8:(base + S) * 8])
                    for q in range(NCHUNK):
                        tb, segT = sb["segs"][q]
                        if segT == 0:
                            continue
                        gather_split(g, tb - base, segT, RG, htabs[q][:, :], ixs)
                    # a_dst window for the sb's blocks  [P, nblk*H] bf16
                    adw = pbg.tile([P, 8 * H], bf16, tag="adw")
                    nc.sync.dma_start(
                        out=adw[:, :nblk * H],
                        in_=bass.AP(sloc, b0 * P * H,
                                    [[H, P], [P * H, nblk], [1, H]]))
                    # O_T: [d, slot] one-hot via PE broadcast of dlocT + is_equal
                    dlT = pbg.tile([1, S * P], bf16, tag="dlT")
                    nc.sync.dma_start(out=dlT[:],
                                      in_=dlocT_d[0:1, base * P:(base + S) * P])
                    oT = pbg.tile([P, S * P], bf16, tag="oT", bufs=1)
                    for st in range(0, S * P, 512):
                        w = min(512, S * P - st)
                        stp = psk.tile([P, 512], f32, tag="stp")
                        nc.tensor.matmul(out=stp[:, :w], lhsT=onek[:],
                                         rhs=dlT[0:1, st:st + w],
                                         start=True, stop=True)
                        nc.vector.tensor_tensor(
                            out=oT[:, st:st + w],
                            in0=iotc[:, 0:1].to_broadcast([P, w]),
                            in1=stp[:, :w],
                            op=mybir.AluOpType.is_equal)
                    # per-edge a_dst via O_T matmuls -> PSUM [P, S*H]
                    pad = psa2.tile([P, S * H], f32, tag="pad")
                    for bi, b in enumerate(sb["blocks"]):
                        for (tg, tt) in sb["runs"][b]:
                            for t in range(tt):
                                rel = tg - base + t
                                nc.tensor.matmul(
                                    out=pad[:, rel * H:(rel + 1) * H],
                                    lhsT=oT[:, rel * P:(rel + 1) * P],
                                    rhs=adw[:, bi * H:(bi + 1) * H],
                                    start=True, stop=True,
                                    skip_group_check=True)
                    # ex = exp(lrelu(asrc+adst)) for all slots  [P, S*H] f32
                    ex = pbb.tile([P, S * H], f32, tag="ex", bufs=1)
                    nc.vector.tensor_tensor(
                        out=ex[:].rearrange("p (t h) -> p t h", t=S),
                        in0=_sub(g[:], HC, [[RG, S], [1, H]]),
                        in1=pad[:].rearrange("p (t h) -> p t h", t=S),
                        op=mybir.AluOpType.add)
                    tmp = pbb.tile([P, S * H], f32, tag="tmp", bufs=1)
                    nc.vector.tensor_scalar_mul(out=tmp[:], in0=ex[:], scalar1=NEG)
                    nc.vector.tensor_tensor(out=ex[:], in0=ex[:], in1=tmp[:],
                                            op=mybir.AluOpType.max)
                    exb = pbb.tile([P, S * H], bf16, tag="exb", bufs=1)
                    nc.scalar.activation(out=exb[:], in_=ex[:],
                                         func=mybir.ActivationFunctionType.Exp)
                    # msg in-place: cols 0:HC *= ex ; cols HC:HC+H = ex
                    nc.vector.tensor_tensor(
                        out=_sub(g[:], 0, [[RG, S], [C, H], [1, C]]),
                        in0=_sub(g[:], 0, [[RG, S], [C, H], [1, C]]),
                        in1=_sub(exb[:], 0, [[H, S], [1, H], [0, C]]),
                        op=mybir.AluOpType.mult)
                    nc.vector.tensor_copy(
                        out=_sub(g[:], HC, [[RG, S], [1, H]]),
                        in_=exb[:].rearrange("p (t h) -> p t h", t=S))
                    # one-hot for all slots  [P, S*P] bf16
                    oh = pbb.tile([P, S * P], bf16, tag="oh", bufs=1)
                    nc.vector.tensor_tensor(
                        out=oh[:].rearrange("p (t q) -> p t q", t=S),
                        in0=_sub(dlc[:], base, [[1, S], [0, P]]),
                        in1=_sub(iot[:], 0, [[0, S], [1, P]]),
                        op=mybir.AluOpType.is_equal)
                    # per-block accumulation + normalize + L2 prep
                    h2w = pbb.tile([P, 8 * R2], f32, tag="h2w")
                    for bi, b in enumerate(sb["blocks"]):
                        runs = sb["runs"][b]
                        ntile = sum(t for _, t in runs)
                        pso = psb.tile([P, RUSE], f32, tag="pso")
                        ti = 0
                        for (tg, tt) in runs:
                            for t in range(tt):
                                rel = tg - base + t
                                nc.tensor.matmul(
                                    out=pso[:],
                                    lhsT=oh[:, rel * P:(rel + 1) * P],
                                    rhs=g[:, rel * RG:rel * RG + RUSE],
                                    start=(ti == 0), stop=(ti == ntile - 1))
                                ti += 1
                        den = pbb.tile([P, H], f32, tag="den")
                        nc.vector.tensor_scalar_max(out=den[:],
                                                    in0=pso[:, HC:HC + H],
                                                    scalar1=1e-20)
                        rde = pbb.tile([P, H], f32, tag="rde")
                        nc.vector.reciprocal(out=rde[:], in_=den[:])
                        o1 = pbb.tile([P, HC], bf16, tag="o1")
                        for hh in range(H):
                            nc.vector.tensor_scalar_mul(
                                out=o1[:, hh * C:(hh + 1) * C],
                                in0=pso[:, hh * C:(hh + 1) * C],
                                scalar1=rde[:, hh:hh + 1])
                        nc.vector.tensor_tensor(out=o1[:], in0=o1[:], in1=b1s[:],
                                                op=mybir.AluOpType.add)
                        nc.vector.tensor_scalar_max(out=o1[:], in0=o1[:],
                                                    scalar1=0.0)
                        ph2 = psh.tile([P, R2], f32, tag="ph2")
                        for k in range(NCK):
                            kk = min(P, HC - k * P)
                            ptr = pst.tile([P, P], bf16, tag="ptr")
                            nc.tensor.transpose(out=ptr[:kk, :],
                                                in_=o1[:, k * P:k * P + kk],
                                                identity=idn[:])
                            rT = pbb.tile([P, P], bf16, tag="rT")
                            nc.vector.tensor_copy(out=rT[:kk, :], in_=ptr[:kk, :])
                            nc.tensor.matmul(out=ph2[:], lhsT=rT[:kk, :],
                                             rhs=w2s[k][:kk, :],
                                             start=(k == 0), stop=(k == NCK - 1))
                        nc.vector.tensor_copy(out=h2w[:, bi * R2:(bi + 1) * R2],
                                              in_=ph2[:])
                    nc.sync.dma_start(
                        out=bass.AP(h2loc, b0 * P * R2,
                                    [[R2, P], [P * R2, nblk], [1, R2]]),
                        in_=h2w[:, :nblk * R2].rearrange(
                            "p (g r) -> p g r", g=nblk))

            # ---------------- AllGather + repack -----------------------------
            nc.gpsimd.collective_compute(
                "AllGather", mybir.AluOpType.bypass,
                replica_groups=[list(range(NC))],
                ins=[h2loc[0:NPC, :]], outs=[h2tab[:, :]])
            # repack [N, R2] -> 256B rows [N, RL2]
            for r in range(NC):
                nc.sync.dma_start(
                    out=bass.AP(h2tabp, r * NPC * RL2, [[RL2, NPC], [1, R2]]),
                    in_=h2tab[r * NPC:(r + 1) * NPC, :])

            # ---------------- Phase C: L2 edge pass --------------------------
            with tc.tile_pool(name="pcg", bufs=2) as pcg, \
                 tc.tile_pool(name="pcb", bufs=2) as pcb, \
                 tc.tile_pool(name="psc", bufs=2, space="PSUM") as psc, \
                 tc.tile_pool(name="psk2", bufs=2, space="PSUM") as psk2, \
                 tc.tile_pool(name="psd2", bufs=2, space="PSUM") as psd2:
                for sb in sb_meta:
                    base, S = sb["base"], sb["S"]
                    nblk = len(sb["blocks"])
                    b0 = sb["blocks"][0]
                    g2 = pcg.tile([P, S * RL2], f32, tag="g2")
                    ixs = pcg.tile([P, S * 8], i16, tag="ixs2")
                    nc.sync.dma_start(out=ixs[:],
                                      in_=ihsrc_d[:, base * 8:(base + S) * 8])
                    for q in range(NCHUNK):
                        tb, segT = sb["segs"][q]
                        if segT == 0:
                            continue
                        hi = N if q == NCHUNK - 1 else (q + 1) * CHB
                        gather_split(g2, tb - base, segT, RL2,
                                     h2tabp[q * CHB:hi, :], ixs)
                    adw2f = pcg.tile([P, 8], f32, tag="adw2f")
                    nc.sync.dma_start(
                        out=adw2f[:, :nblk],
                        in_=bass.AP(h2loc, b0 * P * R2 + CLS + 1,
                                    [[R2, P], [P * R2, nblk], [1, 1]]))
                    adw2 = pcg.tile([P, 8], bf16, tag="adw2")
                    nc.vector.tensor_copy(out=adw2[:, :nblk], in_=adw2f[:, :nblk])
                    dlT = pcg.tile([1, S * P], bf16, tag="dlT2")
                    nc.sync.dma_start(out=dlT[:],
                                      in_=dlocT_d[0:1, base * P:(base + S) * P])
                    oT = pcg.tile([P, S * P], bf16, tag="oT2", bufs=1)
                    for st in range(0, S * P, 512):
                        w = min(512, S * P - st)
                        stp = psk2.tile([P, 512], f32, tag="stp2")
                        nc.tensor.matmul(out=stp[:, :w], lhsT=onek[:],
                                         rhs=dlT[0:1, st:st + w],
                                         start=True, stop=True)
                        nc.vector.tensor_tensor(
                            out=oT[:, st:st + w],
                            in0=iotc[:, 0:1].to_broadcast([P, w]),
                            in1=stp[:, :w],
                            op=mybir.AluOpType.is_equal)
                    pad2 = psd2.tile([P, S], f32, tag="pad2")
                    for bi, b in enumerate(sb["blocks"]):
                        for (tg, tt) in sb["runs"][b]:
                            for t in range(tt):
                                rel = tg - base + t
                                nc.tensor.matmul(
                                    out=pad2[:, rel:rel + 1],
                                    lhsT=oT[:, rel * P:(rel + 1) * P],
                                    rhs=adw2[:, bi:bi + 1],
                                    start=True, stop=True,
                                    skip_group_check=True)
                    ex2 = pcb.tile([P, S], f32, tag="ex2")
                    nc.vector.tensor_tensor(
                        out=ex2[:],
                        in0=_sub(g2[:], CLS, [[RL2, S]]),
                        in1=pad2[:],
                        op=mybir.AluOpType.add)
                    tm2 = pcb.tile([P, S], f32, tag="tm2")
                    nc.vector.tensor_scalar_mul(out=tm2[:], in0=ex2[:], scalar1=NEG)
                    nc.vector.tensor_tensor(out=ex2[:], in0=ex2[:], in1=tm2[:],
                                            op=mybir.AluOpType.max)
                    nc.scalar.activation(out=ex2[:], in_=ex2[:],
                                         func=mybir.ActivationFunctionType.Exp)
                    m2 = pcb.tile([P, S * 3], bf16, tag="m2")
                    nc.vector.tensor_copy(out=_sub(m2[:], CLS, [[3, S]]), in_=ex2[:])
                    nc.vector.tensor_tensor(
                        out=_sub(m2[:], 0, [[3, S], [1, CLS]]),
                        in0=_sub(g2[:], 0, [[RL2, S], [1, CLS]]),
                        in1=_sub(m2[:], CLS, [[3, S], [0, CLS]]),
                        op=mybir.AluOpType.mult)
                    oh2 = pcb.tile([P, S * P], bf16, tag="oh2", bufs=1)
                    nc.vector.tensor_tensor(
                        out=oh2[:].rearrange("p (t q) -> p t q", t=S),
                        in0=_sub(dlc[:], base, [[1, S], [0, P]]),
                        in1=_sub(iot[:], 0, [[0, S], [1, P]]),
                        op=mybir.AluOpType.is_equal)
                    for b in sb["blocks"]:
                        runs = sb["runs"][b]
                        ntile = sum(t for _, t in runs)
                        ps2 = psc.tile([P, 3], f32, tag="ps2")
                        ti = 0
                        for (tg, tt) in runs:
                            for t in range(tt):
                                rel = tg - base + t
                                nc.tensor.matmul(
                                    out=ps2[:],
                                    lhsT=oh2[:, rel * P:(rel + 1) * P],
                                    rhs=m2[:, rel * 3:(rel + 1) * 3],
                                    start=(ti == 0), stop=(ti == ntile - 1))
                                ti += 1
                        den2 = pcb.tile([P, 1], f32, tag="den2")
                        nc.vector.tensor_scalar_max(out=den2[:],
                                                    in0=ps2[:, CLS:CLS + 1],
                                                    scalar1=1e-20)
                        rd2 = pcb.tile([P, 1], f32, tag="rd2")
                        nc.vector.reciprocal(out=rd2[:], in_=den2[:])
                        nc.vector.tensor_scalar_mul(
                            out=vall[:, b * CLS:(b + 1) * CLS],
                            in0=ps2[:, 0:CLS], scalar1=rd2[:, 0:1])
                # batched log-softmax over all blocks:
                # out[:, 2b+i] = v_i - log(exp v_0 + exp v_1)
                #             = -softplus(v_other - v_i)   (CLS == 2)
                nc.vector.tensor_tensor(out=vall[:], in0=vall[:], in1=b2a[:],
                                        op=mybir.AluOpType.add)
                vsw = cp.tile([P, NB * CLS], f32, name="vsw")
                nc.vector.tensor_copy(
                    out=_sub(vsw[:], 0, [[CLS, NB]]),
                    in_=_sub(vall[:], 1, [[CLS, NB]]))
                nc.vector.tensor_copy(
                    out=_sub(vsw[:], 1, [[CLS, NB]]),
                    in_=_sub(vall[:], 0, [[CLS, NB]]))
                nc.vector.tensor_tensor(out=vsw[:], in0=vsw[:], in1=vall[:],
                                        op=mybir.AluOpType.subtract)
                # softplus(d) = ln(1 + exp(d)) via the exp+ln table
                nc.scalar.activation(out=vsw[:], in_=vsw[:],
                                     func=mybir.ActivationFunctionType.Exp)
                nc.vector.tensor_scalar_add(out=vsw[:], in0=vsw[:], scalar1=1.0)
                nc.scalar.activation(out=vsw[:], in_=vsw[:],
                                     func=mybir.ActivationFunctionType.Ln)
                nc.vector.tensor_scalar_mul(out=vsw[:], in0=vsw[:], scalar1=-1.0)
                nfull = NPC // P                  # full blocks
                nc.sync.dma_start(
                    out=bass.AP(out_d, 0, [[CLS, P], [P * CLS, nfull], [1, CLS]]),
                    in_=vsw[:, :nfull * CLS].rearrange(
                        "p (g r) -> p g r", g=nfull))
                rows = NPC - nfull * P
                if rows:
                    nc.sync.dma_start(
                        out=out_d[nfull * P:NPC, :],
                        in_=vsw[:rows, nfull * CLS:(nfull + 1) * CLS])
    nc.finalize()
    return nc


def install_ntff_hook(so_path="/opt/axon/libaxon_pjrt.so"):
    import types
    import ctypes
    import contextlib
    import antenv

    if getattr(antenv, "axon_hooks", None) is not None:
        return
    lib = ctypes.CDLL(so_path)
    if not hasattr(lib, "axon_start_nrt_profile"):
        return
    lib.axon_start_nrt_profile.argtypes = [ctypes.POINTER(ctypes.c_int64),
                                           ctypes.c_size_t]
    lib.axon_start_nrt_profile.restype = ctypes.c_int64
    lib.axon_stop_nrt_profile.argtypes = [ctypes.c_char_p]
    lib.axon_stop_nrt_profile.restype = ctypes.c_int64

    @contextlib.contextmanager
    def _hook(output_dir, device_ids):
        import jax
        jax.devices()
        if device_ids:
            ids = (ctypes.c_int64 * len(device_ids))(*device_ids)
            rc = lib.axon_start_nrt_profile(ids, len(device_ids))
        else:
            rc = lib.axon_start_nrt_profile(None, 0)
        if rc != 0:
            raise RuntimeError(f"axon_start_nrt_profile rc={rc}")
        try:
            yield
        finally:
            n = lib.axon_stop_nrt_profile(str(output_dir).encode())
            print(f"ntff profile: {n} file(s) written to {output_dir}")

    mod = types.ModuleType("antenv.axon_hooks")
    _reg = [_hook]
    mod.set_axon_ntff_profile_hook = lambda h: _reg.__setitem__(0, h)
    mod.get_axon_ntff_profile_hook = lambda: _reg[0]
    sys.modules["antenv.axon_hooks"] = mod
    antenv.axon_hooks = mod


def run(inputs, cfg, trace=False, **kwargs):
    if trace:
        install_ntff_hook()
    in_maps, meta = prep(inputs, cfg)
    nc = build(meta)
    res = bass_utils.run_bass_kernel_spmd(
        nc, in_maps, core_ids=list(range(cfg["NC"])), trace=trace, **kwargs)
    out = np.concatenate([res.results[c]["out"] for c in range(cfg["NC"])], axis=0)
    return out, res


# ----------------------------------------------------------------------------
# harness entry point
# ----------------------------------------------------------------------------

_CFG = dict(N=100000, F=165, H=4, C=64, CLS=2, NC=8)


def kernel(**inputs):
    """Full (unsharded) inputs -> full [N, 2] float32 log-softmax output.

    Shards edges by destination-node range across the 8 NeuronCores,
    compiles and runs the Bass/Tile kernel via run_bass_kernel_spmd,
    and concatenates the per-core output slices.
    """
    out, _ = run(inputs, _CFG, trace=False)
    return np.ascontiguousarray(out.astype(np.float32))


# revision 6
# speedup vs baseline: 1.8292x; 1.2095x over previous
"""GAT 2-layer message-passing network on 8 TRN2 NeuronCores (Bass/Tile).

v4: self-loops handled directly (not as edge slots); dense shared slot
layout — per (superblock, chunk) segment, per-block runs of shared length
maxcnt[b,q] laid back-to-back, one pad-to-128 per segment (12-15% padding
vs 65% in v3). Tiles may span two adjacent dst blocks; two one-hot planes
(iota, iota+128) against a tile-relative dloc make the MM schedule
core-independent. Local node table hloc [NPCp, R1] feeds a_dst windows,
self-loop terms, and the L2 attention stats without core-dependent offsets.

Pipeline:
 - Phase A: htab (4 chunk tensors, 768B rows [h|asrc|adst]) from x@W1aug,
   batched 4 blocks/DMA; hloc for the core's own nodes from xTloc.
 - Phase B: per sb: gather src rows; per-edge a_dst via two-plane
   one-hot-transpose MMs; ex=exp(lrelu(asrc+adst)); msg in gather buffer;
   two-plane one-hot accumulation MMs + self-loop term; normalize, relu;
   h2aug = relu @ W2aug; h2 kept in SBUF (h2all) + h2loc DRAM.
 - AllGather h2loc -> h2tab; repack to 256B rows.
 - Phase C: gather h2 rows; same two-plane scheme; self-loop terms from
   h2all; batched log-softmax via exp+ln; single strided output DMA.
"""
import sys

if "/opt/trn_rl_repo" not in sys.path:
    sys.path.insert(0, "/opt/trn_rl_repo")

import math
import numpy as np
import ml_dtypes

import concourse.bass as bass
import concourse.bacc as bacc
import concourse.mybir as mybir
import concourse.tile as tile
from concourse import bass_utils

P = 128
NEG = 0.2
NCHUNK = 4
NQUEUE = 4
MAXT = 8                 # tiles per dma_gather call
SENT = 1000.0            # sentinel dloc (exact in bf16, never matches iota)

from concourse import tile_sem_assignment as _tsa  # noqa: E402

if not getattr(_tsa.TileClockTick, "_qaware_patched", False):
    _orig_assign_tick = _tsa.TileClockTick._assign_tick

    def _qaware_assign_tick(self, inst):
        if (isinstance(inst, _tsa.DMAInst)
                and inst.engine == mybir.EngineType.Pool):
            self.next_sw_dma_idx = getattr(inst, "queue_num", 0) or 0
        return _orig_assign_tick(self, inst)

    _tsa.TileClockTick._assign_tick = _qaware_assign_tick
    _tsa.TileClockTick._qaware_patched = True


def _wrap16(flat):
    """[n] -> [128, n//16] wrapped in 16 partitions, replicated x8."""
    w = flat.reshape(-1, 16).T
    return np.tile(w, (8, 1))


# ----------------------------------------------------------------------------
# host-side data prep
# ----------------------------------------------------------------------------

def prep(inputs, cfg):
    N, F, H, C, CLS, NC = cfg["N"], cfg["F"], cfg["H"], cfg["C"], cfg["CLS"], cfg["NC"]
    SBG = cfg.get("SBG", 4)
    x = np.asarray(inputs["x"], np.float32)
    ei = np.asarray(inputs["edge_index"])
    W1 = np.asarray(inputs["W1"], np.float32)
    as1 = np.asarray(inputs["att_src1"], np.float32)
    ad1 = np.asarray(inputs["att_dst1"], np.float32)
    b1 = np.asarray(inputs["b1"], np.float32)
    W2 = np.asarray(inputs["W2"], np.float32)
    as2 = np.asarray(inputs["att_src2"], np.float32)
    ad2 = np.asarray(inputs["att_dst2"], np.float32)
    b2 = np.asarray(inputs["b2"], np.float32)

    HC = H * C
    R1 = HC + 2 * H
    RG = 128 * math.ceil(R1 / 128)
    NPC = N // NC
    NB = math.ceil(NPC / P)
    NPCp = NB * P
    NT = (N + P - 1) // P
    Np = NT * P
    CHB = 25088
    assert NT == 782 and 3 * CHB < Np
    assert CHB < 32768 and NPCp < 32768

    # ---- weights / constants -------------------------------------------------
    W1r = W1.reshape(F, H, C)
    Wsrc = np.einsum("fhc,hc->fh", W1r, as1)
    Wdst = np.einsum("fhc,hc->fh", W1r, ad1)
    W1aug = np.concatenate([W1, Wsrc, Wdst], axis=1)          # [F, R1]
    Wsrc2 = W2 @ as2.reshape(CLS, 1)
    Wdst2 = W2 @ ad2.reshape(CLS, 1)
    W2aug = np.concatenate([W2, Wsrc2, Wdst2], axis=1)        # [HC, 4]

    bf16 = ml_dtypes.bfloat16
    xT = np.zeros((F, Np), dtype=bf16)
    xT[:, :N] = x.T.astype(bf16)
    b1rep = np.tile(b1[None, :], (P, 1)).astype(bf16)
    b2all = np.tile(b2[None, :], (P, NB)).astype(np.float32)
    ar = np.arange(P, dtype=np.float32)
    iota2 = np.tile(np.concatenate([ar, ar + P])[None, :], (P, 1)).astype(bf16)
    ident = np.eye(P, dtype=bf16)
    iotac2 = np.stack([ar, ar + P], axis=1)                   # [P, 2] f32

    # ---- edges (no self loops) ----------------------------------------------
    src_all = ei[0].astype(np.int64)
    dst_all = ei[1].astype(np.int64)
    order = np.argsort(dst_all, kind="stable")
    src_s = src_all[order]
    dst_s = dst_all[order]
    # secondary sort by chunk within equal dst not needed; we filter per chunk
    chunk_s = src_s // CHB

    cnts = np.zeros((NC, NB, NCHUNK), np.int64)
    for c in range(NC):
        for b in range(NB):
            base = c * NPC + b * P
            hi = min(base + P, (c + 1) * NPC)
            lo_i = np.searchsorted(dst_s, base)
            hi_i = np.searchsorted(dst_s, hi)
            ch = chunk_s[lo_i:hi_i]
            for q in range(NCHUNK):
                cnts[c, b, q] = (ch == q).sum()
    maxcnt = cnts.max(axis=0)                                 # [NB, NCHUNK]

    # ---- shared slot layout --------------------------------------------------
    sblocks = [list(range(i, min(i + SBG, NB))) for i in range(0, NB, SBG)]
    sb_meta = []
    tile_base = 0
    for blist in sblocks:
        segs = []                 # per q: (tile_base_global, segT)
        run_start = {}            # (b, q) -> slot offset within segment
        sb_base = tile_base
        tiles = []                # per rel tile: list of (bi, plane) covered
        accum = {b: [] for b in blist}
        for q in range(NCHUNK):
            L = 0
            for b in blist:
                run_start[(b, q)] = L
                L += int(maxcnt[b, q])
            segT = math.ceil(L / P)
            segs.append((tile_base, segT))
            for t in range(segT):
                lo, hi = t * P, (t + 1) * P
                cov = [b for b in blist
                       if maxcnt[b, q] > 0
                       and run_start[(b, q)] < hi
                       and run_start[(b, q)] + maxcnt[b, q] > lo]
                assert 1 <= len(cov) <= 2 and cov[-1] - cov[0] == len(cov) - 1
                rel = tile_base + t - sb_base
                gb1 = cov[0]
                tiles.append(dict(rel=rel, gb1=gb1,
                                  mms=[(b - blist[0], b - gb1) for b in cov]))
                for b in cov:
                    accum[b].append((rel, b - gb1))
            tile_base += segT
        sb_meta.append(dict(base=sb_base, S=tile_base - sb_base, segs=segs,
                            blocks=blist, tiles=tiles, accum=accum,
                            run_start=run_start))
    Tsum = tile_base

    # tile gb1 lookup: global tile index -> gb1 (for per-core dloc fill)
    gb1_of = np.zeros(Tsum, np.int64)
    for sb in sb_meta:
        for td in sb["tiles"]:
            gb1_of[sb["base"] + td["rel"]] = td["gb1"]

    # ---- per-core slot tables ------------------------------------------------
    ihsrc_w = np.zeros((NC, P, Tsum * 8), np.int16)
    dloc2d = np.zeros((NC, P, Tsum), bf16)
    dlocT_a = np.zeros((NC, 1, Tsum * P), bf16)
    for c in range(NC):
        ihsrc = np.zeros(Tsum * P, np.int16)
        dloc = np.full(Tsum * P, SENT, np.float32)
        lo = np.searchsorted(dst_s, c * NPC)
        hi = np.searchsorted(dst_s, (c + 1) * NPC)
        cs, cd, cq = src_s[lo:hi], dst_s[lo:hi], chunk_s[lo:hi]
        for sb in sb_meta:
            for q in range(NCHUNK):
                tb, segT = sb["segs"][q]
                seg0 = tb * P
                for b in sb["blocks"]:
                    n = int(cnts[c, b, q])
                    if n == 0:
                        continue
                    base = c * NPC + b * P
                    top = min(base + P, (c + 1) * NPC)
                    s0 = np.searchsorted(cd, base)
                    s1 = np.searchsorted(cd, top)
                    m = cq[s0:s1] == q
                    es, ed = cs[s0:s1][m], cd[s0:s1][m]
                    assert len(es) == n
                    s = seg0 + sb["run_start"][(b, q)]
                    sl = np.arange(s, s + n)
                    ihsrc[sl] = (es - q * CHB).astype(np.int16)
                    dloc[sl] = (ed - c * NPC - gb1_of[sl // P] * P).astype(
                        np.float32)
        assert dloc[dloc != SENT].max(initial=0) < 256
        assert dloc[dloc != SENT].min(initial=0) >= 0
        ihsrc_w[c] = _wrap16(ihsrc)
        dloc2d[c] = dloc.reshape(Tsum, P).T.astype(bf16)
        dlocT_a[c, 0] = dloc.astype(bf16)

    shared = {
        "xT": xT, "W1aug": W1aug.astype(bf16), "W2aug": W2aug.astype(bf16),
        "b1rep": b1rep, "b2all": b2all, "iota2": iota2, "ident": ident,
        "iotac2": iotac2, "onesk": np.ones((1, P), bf16),
    }
    in_maps = []
    for c in range(NC):
        m = dict(shared)
        xl = np.zeros((F, NPCp), dtype=bf16)
        xl[:, :NPC] = xT[:, c * NPC:c * NPC + NPC]
        m["xTloc"] = xl
        m["ihsrc"] = ihsrc_w[c]
        m["dloc2d"] = dloc2d[c]
        m["dlocT"] = dlocT_a[c]
        in_maps.append(m)

    meta = dict(cfg, R1=R1, RG=RG, HC=HC, NPC=NPC, NPCp=NPCp, NB=NB, NT=NT,
                Np=Np, CHB=CHB, Tsum=Tsum, sb_meta=sb_meta, SBG=SBG)
    return in_maps, meta


# ----------------------------------------------------------------------------
# device program
# ----------------------------------------------------------------------------

def _sub(ap, elem_off, dims):
    return bass.AP(ap.tensor, ap.offset + elem_off, [ap.ap[0], *list(dims)])


def build(meta, nc=None):
    N, F, H, C, CLS = meta["N"], meta["F"], meta["H"], meta["C"], meta["CLS"]
    NC, R1, RG, HC = meta["NC"], meta["R1"], meta["RG"], meta["HC"]
    NPC, NPCp, NB, Np = meta["NPC"], meta["NPCp"], meta["NB"], meta["Np"]
    CHB, Tsum = meta["CHB"], meta["Tsum"]
    sb_meta = meta["sb_meta"]
    R2 = CLS + 2
    RL2 = 64
    RUSE = HC + H

    f32, bf16, i16 = mybir.dt.float32, mybir.dt.bfloat16, mybir.dt.int16

    if nc is None:
        nc = bacc.Bacc("TRN2", target_bir_lowering=False, debug=False,
                       num_devices=NC, num_swdge_queues=NQUEUE)

    qrr = [0]

    def gather_split(out_tile, rel, segT, elem, table, ix_tile):
        done = 0
        while done < segT:
            tt = min(MAXT, segT - done)
            r = rel + done
            nc.gpsimd.dma_gather(
                bass.AP(out_tile[:].tensor, out_tile[:].offset + r * elem,
                        [out_tile[:].ap[0], [elem, tt], [1, elem]]),
                table,
                ix_tile[:, r * 8:(r + tt) * 8],
                tt * P, tt * P, elem,
                queue_num=qrr[0] % NQUEUE,
            )
            qrr[0] += 1
            done += tt

    xT_d = nc.dram_tensor("xT", [F, Np], bf16, kind="ExternalInput")
    xTl_d = nc.dram_tensor("xTloc", [F, NPCp], bf16, kind="ExternalInput")
    W1aug_d = nc.dram_tensor("W1aug", [F, R1], bf16, kind="ExternalInput")
    W2aug_d = nc.dram_tensor("W2aug", [HC, R2], bf16, kind="ExternalInput")
    b1rep_d = nc.dram_tensor("b1rep", [P, HC], bf16, kind="ExternalInput")
    b2all_d = nc.dram_tensor("b2all", [P, NB * CLS], f32, kind="ExternalInput")
    iota2_d = nc.dram_tensor("iota2", [P, 2 * P], bf16, kind="ExternalInput")
    ident_d = nc.dram_tensor("ident", [P, P], bf16, kind="ExternalInput")
    ihsrc_d = nc.dram_tensor("ihsrc", [P, Tsum * 8], i16, kind="ExternalInput")
    dloc_d = nc.dram_tensor("dloc2d", [P, Tsum], bf16, kind="ExternalInput")
    dlocT_d = nc.dram_tensor("dlocT", [1, Tsum * P], bf16, kind="ExternalInput")
    iotac2_d = nc.dram_tensor("iotac2", [P, 2], f32, kind="ExternalInput")
    onesk_d = nc.dram_tensor("onesk", [1, P], bf16, kind="ExternalInput")
    out_d = nc.dram_tensor("out", [NPC, CLS], f32, kind="ExternalOutput")

    CH_ROWS = [CHB, CHB, CHB, Np - 3 * CHB]
    htabs = [nc.dram_tensor(f"htab{q}", [CH_ROWS[q], RG], bf16, kind="Internal")
             for q in range(NCHUNK)]
    hloc = nc.dram_tensor("hloc", [NPCp, R1], bf16, kind="Internal")
    h2loc = nc.dram_tensor("h2loc", [NPCp, R2], f32, kind="Internal")
    h2tab = nc.dram_tensor("h2tab", [N, R2], f32, kind="Internal",
                           addr_space="Shared" if NC > 4 else "Local")
    h2tabp = nc.dram_tensor("h2tabp", [N, RL2], f32, kind="Internal")

    FA = min(P, F)
    FB = F - FA
    NCK = (HC + P - 1) // P
    GRP = 4

    with tile.TileContext(nc) as tc:
        with tc.tile_pool(name="const", bufs=1) as cp:
            w1a = cp.tile([FA, R1], bf16)
            nc.sync.dma_start(out=w1a[:], in_=W1aug_d[0:FA, :])
            w1b = cp.tile([FB, R1], bf16)
            nc.sync.dma_start(out=w1b[:], in_=W1aug_d[FA:F, :])
            w2s = []
            for k in range(NCK):
                kk = min(P, HC - k * P)
                w2k = cp.tile([kk, R2], bf16, name=f"w2k{k}")
                nc.sync.dma_start(out=w2k[:], in_=W2aug_d[k * P:k * P + kk, :])
                w2s.append(w2k)
            b1s = cp.tile([P, HC], bf16)
            nc.sync.dma_start(out=b1s[:], in_=b1rep_d[:, :])
            b2a = cp.tile([P, NB * CLS], f32)
            nc.sync.dma_start(out=b2a[:], in_=b2all_d[:, :])
            iot2 = cp.tile([P, 2 * P], bf16)
            nc.sync.dma_start(out=iot2[:], in_=iota2_d[:, :])
            idn = cp.tile([P, P], bf16)
            nc.sync.dma_start(out=idn[:], in_=ident_d[:, :])
            dlc = cp.tile([P, Tsum], bf16)
            nc.sync.dma_start(out=dlc[:], in_=dloc_d[:, :])
            iotc2 = cp.tile([P, 2], f32)
            nc.sync.dma_start(out=iotc2[:], in_=iotac2_d[:, :])
            onek = cp.tile([1, P], bf16)
            nc.sync.dma_start(out=onek[:], in_=onesk_d[:, :])
            vall = cp.tile([P, NB * CLS], f32)
            h2all = cp.tile([P, NB * R2], f32)

            # ---------------- Phase A: feature tables ------------------------
            with tc.tile_pool(name="pa", bufs=3) as pa, \
                 tc.tile_pool(name="psa", bufs=4, space="PSUM") as psa:
                for q in range(NCHUNK):
                    nbq = CH_ROWS[q] // P
                    for g0 in range(0, nbq, GRP):
                        glen = min(GRP, nbq - g0)
                        col0 = q * CHB + g0 * P
                        w = glen * P
                        xa = pa.tile([FA, GRP * P], bf16, tag="xa")
                        nc.sync.dma_start(out=xa[:, :w],
                                          in_=xT_d[0:FA, col0:col0 + w])
                        xb = pa.tile([FB, GRP * P], bf16, tag="xb")
                        nc.sync.dma_start(out=xb[:, :w],
                                          in_=xT_d[FA:F, col0:col0 + w])
                        hsb = pa.tile([P, GRP * R1], bf16, tag="hsb")
                        for k in range(glen):
                            ph = psa.tile([P, R1], f32, tag="ph")
                            nc.tensor.matmul(out=ph[:], lhsT=xa[:, k * P:(k + 1) * P],
                                             rhs=w1a[:], start=True, stop=False)
                            nc.tensor.matmul(out=ph[:], lhsT=xb[:, k * P:(k + 1) * P],
                                             rhs=w1b[:], start=False, stop=True)
                            nc.vector.tensor_copy(out=hsb[:, k * R1:(k + 1) * R1],
                                                  in_=ph[:])
                        nc.sync.dma_start(
                            out=bass.AP(htabs[q], g0 * P * RG,
                                        [[RG, P], [P * RG, glen], [1, R1]]),
                            in_=hsb[:, :glen * R1].rearrange(
                                "p (g r) -> p g r", g=glen))
                # local full rows (a_dst windows + self-loop features)
                for g0 in range(0, NPCp // P, GRP):
                    glen = min(GRP, NPCp // P - g0)
                    w = glen * P
                    xa = pa.tile([FA, GRP * P], bf16, tag="xla")
                    nc.sync.dma_start(out=xa[:, :w],
                                      in_=xTl_d[0:FA, g0 * P:g0 * P + w])
                    xb = pa.tile([FB, GRP * P], bf16, tag="xlb")
                    nc.sync.dma_start(out=xb[:, :w],
                                      in_=xTl_d[FA:F, g0 * P:g0 * P + w])
                    ssb = pa.tile([P, GRP * R1], bf16, tag="ssb")
                    for k in range(glen):
                        ps = psa.tile([P, R1], f32, tag="pss")
                        nc.tensor.matmul(out=ps[:], lhsT=xa[:, k * P:(k + 1) * P],
                                         rhs=w1a[:], start=True, stop=False)
                        nc.tensor.matmul(out=ps[:], lhsT=xb[:, k * P:(k + 1) * P],
                                         rhs=w1b[:], start=False, stop=True)
                        nc.vector.tensor_copy(out=ssb[:, k * R1:(k + 1) * R1],
                                              in_=ps[:])
                    nc.sync.dma_start(
                        out=bass.AP(hloc, g0 * P * R1,
                                    [[R1, P], [P * R1, glen], [1, R1]]),
                        in_=ssb[:, :glen * R1].rearrange(
                            "p (g r) -> p g r", g=glen))

            # ---------------- Phase B: L1 edge pass --------------------------
            with tc.tile_pool(name="pbg", bufs=2) as pbg, \
                 tc.tile_pool(name="pbb", bufs=2) as pbb, \
                 tc.tile_pool(name="psb", bufs=2, space="PSUM") as psb, \
                 tc.tile_pool(name="pst", bufs=1, space="PSUM") as pst, \
                 tc.tile_pool(name="psh", bufs=1, space="PSUM") as psh, \
                 tc.tile_pool(name="psk", bufs=2, space="PSUM") as psk, \
                 tc.tile_pool(name="psa2", bufs=2, space="PSUM") as psa2:
                for sb in sb_meta:
                    base, S = sb["base"], sb["S"]
                    nblk = len(sb["blocks"])
                    b0 = sb["blocks"][0]
                    g = pbg.tile([P, S * RG], bf16, tag="g")
                    ixs = pbg.tile([P, S * 8], i16, tag="ixs")
                    nc.sync.dma_start(out=ixs[:],
                                      in_=ihsrc_d[:, base * 8:(base + S) * 8])
                    for q in range(NCHUNK):
                        tb, segT = sb["segs"][q]
                        if segT == 0:
                            continue
                        gather_split(g, tb - base, segT, RG, htabs[q][:, :], ixs)
                    # local rows window [P, nblk*R1]: h, asrc, adst of own nodes
                    hbl = pbg.tile([P, 4 * R1], bf16, tag="hbl")
                    nc.sync.dma_start(
                        out=hbl[:, :nblk * R1],
                        in_=bass.AP(hloc, b0 * P * R1,
                                    [[R1, P], [P * R1, nblk], [1, R1]]))
                    # O_T planes: [d, slot] one-hots via PE broadcast + is_equal
                    dlT = pbg.tile([1, S * P], bf16, tag="dlT")
                    nc.sync.dma_start(out=dlT[:],
                                      in_=dlocT_d[0:1, base * P:(base + S) * P])
                    oTa = pbg.tile([P, S * P], bf16, tag="oTa", bufs=1)
                    oTb = pbg.tile([P, S * P], bf16, tag="oTb", bufs=1)
                    for st in range(0, S * P, 512):
                        w = min(512, S * P - st)
                        stp = psk.tile([P, 512], f32, tag="stp")
                        nc.tensor.matmul(out=stp[:, :w], lhsT=onek[:],
                                         rhs=dlT[0:1, st:st + w],
                                         start=True, stop=True)
                        nc.vector.tensor_tensor(
                            out=oTa[:, st:st + w],
                            in0=iotc2[:, 0:1].to_broadcast([P, w]),
                            in1=stp[:, :w],
                            op=mybir.AluOpType.is_equal)
                        nc.vector.tensor_tensor(
                            out=oTb[:, st:st + w],
                            in0=iotc2[:, 1:2].to_broadcast([P, w]),
                            in1=stp[:, :w],
                            op=mybir.AluOpType.is_equal)
                    # per-edge a_dst via plane MMs -> PSUM [P, S*H]
                    pad = psa2.tile([P, S * H], f32, tag="pad")
                    for td in sb["tiles"]:
                        rel = td["rel"]
                        nmm = len(td["mms"])
                        for mi, (bi, plane) in enumerate(td["mms"]):
                            oT = oTa if plane == 0 else oTb
                            nc.tensor.matmul(
                                out=pad[:, rel * H:(rel + 1) * H],
                                lhsT=oT[:, rel * P:(rel + 1) * P],
                                rhs=hbl[:, bi * R1 + HC + H:bi * R1 + HC + 2 * H],
                                start=(mi == 0), stop=(mi == nmm - 1),
                                skip_group_check=True)
                    # ex = exp(lrelu(asrc+adst)) for all slots  [P, S*H]
                    ex = pbb.tile([P, S * H], f32, tag="ex", bufs=1)
                    nc.vector.tensor_tensor(
                        out=ex[:].rearrange("p (t h) -> p t h", t=S),
                        in0=_sub(g[:], HC, [[RG, S], [1, H]]),
                        in1=pad[:].rearrange("p (t h) -> p t h", t=S),
                        op=mybir.AluOpType.add)
                    tmp = pbb.tile([P, S * H], f32, tag="tmp", bufs=1)
                    nc.vector.tensor_scalar_mul(out=tmp[:], in0=ex[:], scalar1=NEG)
                    nc.vector.tensor_tensor(out=ex[:], in0=ex[:], in1=tmp[:],
                                            op=mybir.AluOpType.max)
                    exb = pbb.tile([P, S * H], bf16, tag="exb", bufs=1)
                    nc.scalar.activation(out=exb[:], in_=ex[:],
                                         func=mybir.ActivationFunctionType.Exp)
                    # msg in-place: cols 0:HC *= ex ; cols HC:HC+H = ex
                    nc.vector.tensor_tensor(
                        out=_sub(g[:], 0, [[RG, S], [C, H], [1, C]]),
                        in0=_sub(g[:], 0, [[RG, S], [C, H], [1, C]]),
                        in1=_sub(exb[:], 0, [[H, S], [1, H], [0, C]]),
                        op=mybir.AluOpType.mult)
                    nc.vector.tensor_copy(
                        out=_sub(g[:], HC, [[RG, S], [1, H]]),
                        in_=exb[:].rearrange("p (t h) -> p t h", t=S))
                    # one-hot planes [P, S*P]
                    oha = pbb.tile([P, S * P], bf16, tag="oha", bufs=1)
                    nc.vector.tensor_tensor(
                        out=oha[:].rearrange("p (t q) -> p t q", t=S),
                        in0=_sub(dlc[:], base, [[1, S], [0, P]]),
                        in1=_sub(iot2[:], 0, [[0, S], [1, P]]),
                        op=mybir.AluOpType.is_equal)
                    ohb = pbb.tile([P, S * P], bf16, tag="ohb", bufs=1)
                    nc.vector.tensor_tensor(
                        out=ohb[:].rearrange("p (t q) -> p t q", t=S),
                        in0=_sub(dlc[:], base, [[1, S], [0, P]]),
                        in1=_sub(iot2[:], P, [[0, S], [1, P]]),
                        op=mybir.AluOpType.is_equal)
                    # self-loop stats for the sb's blocks  [P, nblk*H]
                    exs = pbb.tile([P, 4 * H], f32, tag="exs")
                    nc.vector.tensor_tensor(
                        out=exs[:, :nblk * H].rearrange("p (b h) -> p b h", b=nblk),
                        in0=_sub(hbl[:], HC, [[R1, nblk], [1, H]]),
                        in1=_sub(hbl[:], HC + H, [[R1, nblk], [1, H]]),
                        op=mybir.AluOpType.add)
                    tms = pbb.tile([P, 4 * H], f32, tag="tms")
                    nc.vector.tensor_scalar_mul(out=tms[:, :nblk * H],
                                                in0=exs[:, :nblk * H], scalar1=NEG)
                    nc.vector.tensor_tensor(out=exs[:, :nblk * H],
                                            in0=exs[:, :nblk * H],
                                            in1=tms[:, :nblk * H],
                                            op=mybir.AluOpType.max)
                    exsb = pbb.tile([P, 4 * H], bf16, tag="exsb")
                    nc.scalar.activation(out=exsb[:, :nblk * H],
                                         in_=exs[:, :nblk * H],
                                         func=mybir.ActivationFunctionType.Exp)
                    # per-block accumulation + normalize + L2 prep
                    h2w = pbb.tile([P, 8 * R2], f32, tag="h2w")
                    for bi, b in enumerate(sb["blocks"]):
                        mms = sb["accum"][b]
                        pso = psb.tile([P, RUSE], f32, tag="pso")
                        for mi, (rel, plane) in enumerate(mms):
                            oh = oha if plane == 0 else ohb
                            nc.tensor.matmul(
                                out=pso[:],
                                lhsT=oh[:, rel * P:(rel + 1) * P],
                                rhs=g[:, rel * RG:rel * RG + RUSE],
                                start=(mi == 0), stop=(mi == len(mms) - 1))
                        # self-loop message + denominator
                        tmb = pbb.tile([P, HC], bf16, tag="tmb")
                        nc.vector.tensor_tensor(
                            out=tmb[:].rearrange("p (h c) -> p h c", h=H),
                            in0=_sub(hbl[:], bi * R1, [[C, H], [1, C]]),
                            in1=_sub(exsb[:], bi * H, [[1, H], [0, C]]),
                            op=mybir.AluOpType.mult)
                        o1p = pbb.tile([P, HC], f32, tag="o1p")
                        nc.vector.tensor_tensor(out=o1p[:], in0=pso[:, 0:HC],
                                                in1=tmb[:],
                                                op=mybir.AluOpType.add)
                        den = pbb.tile([P, H], f32, tag="den")
                        nc.vector.tensor_tensor(
                            out=den[:], in0=pso[:, HC:HC + H],
                            in1=exsb[:, bi * H:(bi + 1) * H],
                            op=mybir.AluOpType.add)
                        rde = pbb.tile([P, H], f32, tag="rde")
                        nc.vector.reciprocal(out=rde[:], in_=den[:])
                        o1 = pbb.tile([P, HC], bf16, tag="o1")
                        for hh in range(H):
                            nc.vector.tensor_scalar_mul(
                                out=o1[:, hh * C:(hh + 1) * C],
                                in0=o1p[:, hh * C:(hh + 1) * C],
                                scalar1=rde[:, hh:hh + 1])
                        nc.vector.tensor_tensor(out=o1[:], in0=o1[:], in1=b1s[:],
                                                op=mybir.AluOpType.add)
                        nc.vector.tensor_scalar_max(out=o1[:], in0=o1[:],
                                                    scalar1=0.0)
                        ph2 = psh.tile([P, R2], f32, tag="ph2")
                        for k in range(NCK):
                            kk = min(P, HC - k * P)
                            ptr = pst.tile([P, P], bf16, tag="ptr")
                            nc.tensor.transpose(out=ptr[:kk, :],
                                                in_=o1[:, k * P:k * P + kk],
                                                identity=idn[:])
                            rT = pbb.tile([P, P], bf16, tag="rT")
                            nc.vector.tensor_copy(out=rT[:kk, :], in_=ptr[:kk, :])
                            nc.tensor.matmul(out=ph2[:], lhsT=rT[:kk, :],
                                             rhs=w2s[k][:kk, :],
                                             start=(k == 0), stop=(k == NCK - 1))
                        nc.vector.tensor_copy(out=h2w[:, bi * R2:(bi + 1) * R2],
                                              in_=ph2[:])
                    nc.vector.tensor_copy(
                        out=h2all[:, b0 * R2:(b0 + nblk) * R2],
                        in_=h2w[:, :nblk * R2])
                    nc.sync.dma_start(
                        out=bass.AP(h2loc, b0 * P * R2,
                                    [[R2, P], [P * R2, nblk], [1, R2]]),
                        in_=h2w[:, :nblk * R2].rearrange(
                            "p (g r) -> p g r", g=nblk))

            # ---------------- AllGather + repack -----------------------------
            nc.gpsimd.collective_compute(
                "AllGather", mybir.AluOpType.bypass,
                replica_groups=[list(range(NC))],
                ins=[h2loc[0:NPC, :]], outs=[h2tab[:, :]])
            for r in range(NC):
                nc.sync.dma_start(
                    out=bass.AP(h2tabp, r * NPC * RL2, [[RL2, NPC], [1, R2]]),
                    in_=h2tab[r * NPC:(r + 1) * NPC, :])

            # ---------------- Phase C: L2 edge pass --------------------------
            with tc.tile_pool(name="pcg", bufs=2) as pcg, \
                 tc.tile_pool(name="pcb", bufs=2) as pcb, \
                 tc.tile_pool(name="psc", bufs=2, space="PSUM") as psc, \
                 tc.tile_pool(name="psk2", bufs=2, space="PSUM") as psk2, \
                 tc.tile_pool(name="psd2", bufs=2, space="PSUM") as psd2:
                for sb in sb_meta:
                    base, S = sb["base"], sb["S"]
                    nblk = len(sb["blocks"])
                    b0 = sb["blocks"][0]
                    g2 = pcg.tile([P, S * RL2], f32, tag="g2")
                    ixs = pcg.tile([P, S * 8], i16, tag="ixs2")
                    nc.sync.dma_start(out=ixs[:],
                                      in_=ihsrc_d[:, base * 8:(base + S) * 8])
                    for q in range(NCHUNK):
                        tb, segT = sb["segs"][q]
                        if segT == 0:
                            continue
                        hi = N if q == NCHUNK - 1 else (q + 1) * CHB
                        gather_split(g2, tb - base, segT, RL2,
                                     h2tabp[q * CHB:hi, :], ixs)
                    # a_dst2 window from resident h2all
                    adw2 = pcg.tile([P, 8], bf16, tag="adw2")
                    nc.vector.tensor_copy(
                        out=adw2[:, :nblk],
                        in_=_sub(h2all[:], b0 * R2 + CLS + 1, [[R2, nblk]]))
                    dlT = pcg.tile([1, S * P], bf16, tag="dlT2")
                    nc.sync.dma_start(out=dlT[:],
                                      in_=dlocT_d[0:1, base * P:(base + S) * P])
                    oTa = pcg.tile([P, S * P], bf16, tag="oT2a", bufs=1)
                    oTb = pcg.tile([P, S * P], bf16, tag="oT2b", bufs=1)
                    for st in range(0, S * P, 512):
                        w = min(512, S * P - st)
                        stp = psk2.tile([P, 512], f32, tag="stp2")
                        nc.tensor.matmul(out=stp[:, :w], lhsT=onek[:],
                                         rhs=dlT[0:1, st:st + w],
                                         start=True, stop=True)
                        nc.vector.tensor_tensor(
                            out=oTa[:, st:st + w],
                            in0=iotc2[:, 0:1].to_broadcast([P, w]),
                            in1=stp[:, :w],
                            op=mybir.AluOpType.is_equal)
                        nc.vector.tensor_tensor(
                            out=oTb[:, st:st + w],
                            in0=iotc2[:, 1:2].to_broadcast([P, w]),
                            in1=stp[:, :w],
                            op=mybir.AluOpType.is_equal)
                    pad2 = psd2.tile([P, S], f32, tag="pad2")
                    for td in sb["tiles"]:
                        rel = td["rel"]
                        nmm = len(td["mms"])
                        for mi, (bi, plane) in enumerate(td["mms"]):
                            oT = oTa if plane == 0 else oTb
                            nc.tensor.matmul(
                                out=pad2[:, rel:rel + 1],
                                lhsT=oT[:, rel * P:(rel + 1) * P],
                                rhs=adw2[:, bi:bi + 1],
                                start=(mi == 0), stop=(mi == nmm - 1),
                                skip_group_check=True)
                    ex2 = pcb.tile([P, S], f32, tag="ex2")
                    nc.vector.tensor_tensor(
                        out=ex2[:],
                        in0=_sub(g2[:], CLS, [[RL2, S]]),
                        in1=pad2[:],
                        op=mybir.AluOpType.add)
                    tm2 = pcb.tile([P, S], f32, tag="tm2")
                    nc.vector.tensor_scalar_mul(out=tm2[:], in0=ex2[:], scalar1=NEG)
                    nc.vector.tensor_tensor(out=ex2[:], in0=ex2[:], in1=tm2[:],
                                            op=mybir.AluOpType.max)
                    nc.scalar.activation(out=ex2[:], in_=ex2[:],
                                         func=mybir.ActivationFunctionType.Exp)
                    m2 = pcb.tile([P, S * 3], bf16, tag="m2")
                    nc.vector.tensor_copy(out=_sub(m2[:], CLS, [[3, S]]), in_=ex2[:])
                    nc.vector.tensor_tensor(
                        out=_sub(m2[:], 0, [[3, S], [1, CLS]]),
                        in0=_sub(g2[:], 0, [[RL2, S], [1, CLS]]),
                        in1=_sub(m2[:], CLS, [[3, S], [0, CLS]]),
                        op=mybir.AluOpType.mult)
                    oha = pcb.tile([P, S * P], bf16, tag="oh2a", bufs=1)
                    nc.vector.tensor_tensor(
                        out=oha[:].rearrange("p (t q) -> p t q", t=S),
                        in0=_sub(dlc[:], base, [[1, S], [0, P]]),
                        in1=_sub(iot2[:], 0, [[0, S], [1, P]]),
                        op=mybir.AluOpType.is_equal)
                    ohb = pcb.tile([P, S * P], bf16, tag="oh2b", bufs=1)
                    nc.vector.tensor_tensor(
                        out=ohb[:].rearrange("p (t q) -> p t q", t=S),
                        in0=_sub(dlc[:], base, [[1, S], [0, P]]),
                        in1=_sub(iot2[:], P, [[0, S], [1, P]]),
                        op=mybir.AluOpType.is_equal)
                    # self-loop L2 stats [P, nblk]
                    ex2s = pcb.tile([P, 8], f32, tag="ex2s")
                    nc.vector.tensor_tensor(
                        out=ex2s[:, :nblk],
                        in0=_sub(h2all[:], b0 * R2 + CLS, [[R2, nblk]]),
                        in1=_sub(h2all[:], b0 * R2 + CLS + 1, [[R2, nblk]]),
                        op=mybir.AluOpType.add)
                    tm2s = pcb.tile([P, 8], f32, tag="tm2s")
                    nc.vector.tensor_scalar_mul(out=tm2s[:, :nblk],
                                                in0=ex2s[:, :nblk], scalar1=NEG)
                    nc.vector.tensor_tensor(out=ex2s[:, :nblk], in0=ex2s[:, :nblk],
                                            in1=tm2s[:, :nblk],
                                            op=mybir.AluOpType.max)
                    nc.scalar.activation(out=ex2s[:, :nblk], in_=ex2s[:, :nblk],
                                         func=mybir.ActivationFunctionType.Exp)
                    for bi, b in enumerate(sb["blocks"]):
                        mms = sb["accum"][b]
                        ps2 = psc.tile([P, 3], f32, tag="ps2")
                        for mi, (rel, plane) in enumerate(mms):
                            oh = oha if plane == 0 else ohb
                            nc.tensor.matmul(
                                out=ps2[:],
                                lhsT=oh[:, rel * P:(rel + 1) * P],
                                rhs=m2[:, rel * 3:(rel + 1) * 3],
                                start=(mi == 0), stop=(mi == len(mms) - 1))
                        tmp2 = pcb.tile([P, CLS], f32, tag="tmp2")
                        nc.vector.tensor_tensor(
                            out=tmp2[:],
                            in0=_sub(h2all[:], b * R2, [[1, CLS]]),
                            in1=ex2s[:, bi:bi + 1].to_broadcast([P, CLS]),
                            op=mybir.AluOpType.mult)
                        v0 = pcb.tile([P, CLS], f32, tag="v0")
                        nc.vector.tensor_tensor(out=v0[:], in0=ps2[:, 0:CLS],
                                                in1=tmp2[:],
                                                op=mybir.AluOpType.add)
                        den2 = pcb.tile([P, 1], f32, tag="den2")
                        nc.vector.tensor_tensor(out=den2[:], in0=ps2[:, CLS:CLS + 1],
                                                in1=ex2s[:, bi:bi + 1],
                                                op=mybir.AluOpType.add)
                        rd2 = pcb.tile([P, 1], f32, tag="rd2")
                        nc.vector.reciprocal(out=rd2[:], in_=den2[:])
                        nc.vector.tensor_scalar_mul(
                            out=vall[:, b * CLS:(b + 1) * CLS],
                            in0=v0[:], scalar1=rd2[:, 0:1])
                # batched log-softmax: out[:,2b+i] = -ln(1+exp(v_other-v_i))
                nc.vector.tensor_tensor(out=vall[:], in0=vall[:], in1=b2a[:],
                                        op=mybir.AluOpType.add)
                vsw = cp.tile([P, NB * CLS], f32, name="vsw")
                nc.vector.tensor_copy(
                    out=_sub(vsw[:], 0, [[CLS, NB]]),
                    in_=_sub(vall[:], 1, [[CLS, NB]]))
                nc.vector.tensor_copy(
                    out=_sub(vsw[:], 1, [[CLS, NB]]),
                    in_=_sub(vall[:], 0, [[CLS, NB]]))
                nc.vector.tensor_tensor(out=vsw[:], in0=vsw[:], in1=vall[:],
                                        op=mybir.AluOpType.subtract)
                nc.scalar.activation(out=vsw[:], in_=vsw[:],
                                     func=mybir.ActivationFunctionType.Exp)
                nc.vector.tensor_scalar_add(out=vsw[:], in0=vsw[:], scalar1=1.0)
                nc.scalar.activation(out=vsw[:], in_=vsw[:],
                                     func=mybir.ActivationFunctionType.Ln)
                nc.vector.tensor_scalar_mul(out=vsw[:], in0=vsw[:], scalar1=-1.0)
                nfull = NPC // P
                nc.sync.dma_start(
                    out=bass.AP(out_d, 0, [[CLS, P], [P * CLS, nfull], [1, CLS]]),
                    in_=vsw[:, :nfull * CLS].rearrange(
                        "p (g r) -> p g r", g=nfull))
                rows = NPC - nfull * P
                if rows:
                    nc.sync.dma_start(
                        out=out_d[nfull * P:NPC, :],
                        in_=vsw[:rows, nfull * CLS:(nfull + 1) * CLS])
    nc.finalize()
    return nc


def install_ntff_hook(so_path="/opt/axon/libaxon_pjrt.so"):
    import types
    import ctypes
    import contextlib
    import antenv

    if getattr(antenv, "axon_hooks", None) is not None:
        return
    lib = ctypes.CDLL(so_path)
    if not hasattr(lib, "axon_start_nrt_profile"):
        return
    lib.axon_start_nrt_profile.argtypes = [ctypes.POINTER(ctypes.c_int64),
                                           ctypes.c_size_t]
    lib.axon_start_nrt_profile.restype = ctypes.c_int64
    lib.axon_stop_nrt_profile.argtypes = [ctypes.c_char_p]
    lib.axon_stop_nrt_profile.restype = ctypes.c_int64

    @contextlib.contextmanager
    def _hook(output_dir, device_ids):
        import jax
        jax.devices()
        if device_ids:
            ids = (ctypes.c_int64 * len(device_ids))(*device_ids)
            rc = lib.axon_start_nrt_profile(ids, len(device_ids))
        else:
            rc = lib.axon_start_nrt_profile(None, 0)
        if rc != 0:
            raise RuntimeError(f"axon_start_nrt_profile rc={rc}")
        try:
            yield
        finally:
            n = lib.axon_stop_nrt_profile(str(output_dir).encode())
            print(f"ntff profile: {n} file(s) written to {output_dir}")

    mod = types.ModuleType("antenv.axon_hooks")
    _reg = [_hook]
    mod.set_axon_ntff_profile_hook = lambda h: _reg.__setitem__(0, h)
    mod.get_axon_ntff_profile_hook = lambda: _reg[0]
    sys.modules["antenv.axon_hooks"] = mod
    antenv.axon_hooks = mod


def run(inputs, cfg, trace=False, **kwargs):
    if trace:
        install_ntff_hook()
    in_maps, meta = prep(inputs, cfg)
    nc = build(meta)
    res = bass_utils.run_bass_kernel_spmd(
        nc, in_maps, core_ids=list(range(cfg["NC"])), trace=trace, **kwargs)
    out = np.concatenate([res.results[c]["out"] for c in range(cfg["NC"])], axis=0)
    return out, res


_CFG = dict(N=100000, F=165, H=4, C=64, CLS=2, NC=8)


def kernel(**inputs):
    """Full (unsharded) inputs -> full [N, 2] float32 log-softmax output."""
    out, _ = run(inputs, _CFG, trace=False)
    return np.ascontiguousarray(out.astype(np.float32))
